# revision 1
# baseline (speedup 1.0000x reference)
"""Block-causal GQA attention for Trainium2, 8 NeuronCores.

Sharding: core = (batch b, GQA group g): 2 batches x 4 kv-groups.
Each core computes its 4 q-heads + 1 kv-head on one batch element in a
"transposed" layout (head_dim on partitions, tokens on free dim), then a
row-parallel partial out-projection; the host sums the 4 partials per batch.

Layout/algebra notes:
- All matmuls run in float32r (full-rate fp32 on the PE at free-dim >= 256).
- RMSNorm weights + attention scale are folded into host-built RoPE tables;
  rotate_half becomes a partition-block swap (sign folded into the sin table).
- 1/rms factors are applied via gpsimd partition_broadcast + one DVE multiply.
- Softmax: scores never need a running max (|s| bounded via host-computed C
  shift); exp on ScalarE reads PSUM directly; denominator comes free as a
  65th ones-row on V in the PV matmul; normalization fuses into the PSUM
  evacuation multiply.
- The attention mask is analyzed on the host into a per-128x128-tile
  schedule (skip / full / mixed); mixed tiles multiply 0/1 tiles on GpSimd.
"""
import sys
import types
import numpy as np
import ml_dtypes

B, S, DIM = 2, 2048, 1024
H, KVH, HD = 16, 4, 64
EPS = 1e-6
SCALE = HD ** -0.5
PT_TILES = S // 128  # 16
N_CHUNK = 512
N_CHUNKS = S // N_CHUNK  # 4

_BUILD_CACHE = {}
_BLOCKIND = np.zeros((2, 128), np.float32)
_BLOCKIND[0, 0:64] = 1.0
_BLOCKIND[1, 64:128] = 1.0


def _analyze_mask(mask):
    """Classify 128x128 tiles: 0=skip, 1=full, 2=mixed. Returns status grid,
    mixed tile stack (transposed to (k,q) layout, 0/1 float32), and index map.
    Index 0 of the stack is always the all-zero tile."""
    T = PT_TILES
    status = np.zeros((T, T), np.int8)
    tiles = [np.zeros((128, 128), np.float32)]
    idx = {}
    m = np.asarray(mask)
    for i in range(T):
        for j in range(T):
            sub = m[i * 128:(i + 1) * 128, j * 128:(j + 1) * 128]
            if not sub.any():
                status[i, j] = 0
            elif sub.all():
                status[i, j] = 1
            else:
                status[i, j] = 2
                idx[(i, j)] = len(tiles)
                tiles.append(np.ascontiguousarray(sub.T).astype(np.float32))
    return status, np.stack(tiles), idx


def _make_schedule(status, idx):
    """Per chunk: list of (ktile j, s0, s1, [(subtile s, mask_tile_index)])
    where [s0*128, s1*128) is the contiguous span of alive q-subtiles and the
    list holds per-subtile multiplies (zero tile for dead-in-span, mixed id
    for partial)."""
    sched = []
    for ci in range(N_CHUNKS):
        qts = list(range(4 * ci, 4 * ci + 4))
        entries = []
        for j in range(PT_TILES):
            st = [status[i, j] for i in qts]
            if not any(st):
                continue
            alive = [s for s in range(4) if st[s] != 0]
            s0, s1 = alive[0], alive[-1] + 1
            mults = []
            for s in range(s0, s1):
                if st[s] == 1:
                    continue
                mults.append((s, 0 if st[s] == 0 else idx[(qts[s], j)]))
            entries.append((j, s0, s1, mults))
        sched.append(entries)
    return sched


def _build(sched_key, sched, n_masks, neg_c):
    import concourse.bacc as bacc
    import concourse.mybir as mybir
    import concourse.tile as tile
    from concourse.masks import make_identity

    F32 = mybir.dt.float32
    F32R = mybir.dt.float32r
    BF16 = mybir.dt.bfloat16

    nc = bacc.Bacc("TRN2", target_bir_lowering=False, debug=False)
    xT = nc.dram_tensor("xT", (DIM, S), F32R, kind="ExternalInput").ap()
    wq = nc.dram_tensor("wq", (DIM, 256), F32R, kind="ExternalInput").ap()
    wkv = nc.dram_tensor("wkv", (DIM, 128), F32R, kind="ExternalInput").ap()
    wo = nc.dram_tensor("wo", (256, DIM), BF16, kind="ExternalInput").ap()
    cosq = nc.dram_tensor("cosq", (128, S), F32, kind="ExternalInput").ap()
    sinq = nc.dram_tensor("sinq", (128, S), F32, kind="ExternalInput").ap()
    cosk = nc.dram_tensor("cosk", (64, S), F32, kind="ExternalInput").ap()
    sink = nc.dram_tensor("sink", (64, S), F32, kind="ExternalInput").ap()
    masks = nc.dram_tensor("masks", (n_masks, 128, 128), BF16,
                           kind="ExternalInput").ap()
    blockind_d = nc.dram_tensor("blockind", (2, 128), F32R,
                                kind="ExternalInput").ap()
    outT = nc.dram_tensor("outT", (DIM, S), F32, kind="ExternalOutput").ap()

    with tile.TileContext(nc) as tc:
        with tc.tile_pool(name="persist", bufs=1) as pp:
            # --- persistent tiles -------------------------------------
            wq_sb = pp.tile([128, 8, 256], F32R)
            nc.sync.dma_start(out=wq_sb, in_=wq.rearrange("(k p) m -> p k m", p=128))
            wkv_sb = pp.tile([128, 8, 128], F32R)
            nc.sync.dma_start(out=wkv_sb, in_=wkv.rearrange("(k p) m -> p k m", p=128))
            masks_sb = pp.tile([128, n_masks, 128], BF16)
            nc.sync.dma_start(out=masks_sb, in_=masks.rearrange("n k q -> k n q"))
            blockind = pp.tile([2, 128], F32R)
            nc.sync.dma_start(out=blockind[:], in_=blockind_d)
            wo_sb = pp.tile([128, 2, DIM], BF16)
            nc.sync.dma_start(out=wo_sb, in_=wo.rearrange("(k p) m -> p k m", p=128))

            t1a = [pp.tile([128, S], F32, tag=f"t1a{m}", name=f"t1a{m}") for m in range(2)]
            nrq = [pp.tile([2, S], F32R, tag=f"nrq{m}", name=f"nrq{m}") for m in range(2)]
            kt2 = pp.tile([128, S], F32R)
            v_aug = pp.tile([128, PT_TILES, 65], BF16)
            rkT = pp.tile([128, 2 * PT_TILES], F32)

            ones1 = pp.tile([128, 1], F32)
            nc.vector.memset(ones1, 1.0)
            nc.vector.tensor_copy(v_aug[:, :, 64:65],
                                  ones1[:].broadcast_to([128, PT_TILES, 1]))
            oq_f = pp.tile([128, 2], F32)
            nc.vector.memset(oq_f, 0.0)
            nc.vector.memset(oq_f[0:64, 0:1], 1.0)
            nc.vector.memset(oq_f[64:128, 1:2], 1.0)
            onesq = pp.tile([128, 2], F32R)
            nc.vector.tensor_copy(onesq[:], oq_f[:])
            ok_f = pp.tile([64, 2], F32)
            nc.vector.memset(ok_f, 1.0)
            onesk = pp.tile([64, 2], F32R)
            nc.vector.tensor_copy(onesk[:], ok_f[:])
            ident = pp.tile([64, 64], F32)
            make_identity(nc, ident[:])
            eps2 = pp.tile([2, 1], F32)
            nc.vector.memset(eps2, EPS)
            eps128 = pp.tile([128, 1], F32)
            nc.vector.memset(eps128, EPS)
            bias_c = pp.tile([128, 1], F32)
            nc.vector.memset(bias_c, neg_c)

            # ============ single scope: all pools live together ========
            with tc.tile_pool(name="p1", bufs=2) as p1, \
                 tc.tile_pool(name="p2", bufs=8) as p2, \
                 tc.tile_pool(name="p2s", bufs=2) as p2s, \
                 tc.tile_pool(name="p3", bufs=2) as p3, \
                 tc.tile_pool(name="pst", bufs=2, space="PSUM") as pst, \
                 tc.tile_pool(name="psv", bufs=4, space="PSUM") as psv:
                ps1 = pst
                ps1b = pst

                def phase1a(ci):
                    off = ci * N_CHUNK
                    xt = p1.tile([128, 8, N_CHUNK], F32R, tag="xt", name=f"xt{ci}")
                    nc.sync.dma_start(
                        out=xt,
                        in_=xT[:, off:off + N_CHUNK].rearrange("(k p) n -> p k n", p=128))
                    cq = p1.tile([128, N_CHUNK], F32, tag="cq", name=f"cq{ci}")
                    nc.sync.dma_start(out=cq, in_=cosq[:, off:off + N_CHUNK])
                    sq = p1.tile([128, N_CHUNK], F32, tag="sq", name=f"sq{ci}")
                    nc.sync.dma_start(out=sq, in_=sinq[:, off:off + N_CHUNK])
                    ck = p1.tile([64, N_CHUNK], F32, tag="ck", name=f"ck{ci}")
                    nc.sync.dma_start(out=ck, in_=cosk[:, off:off + N_CHUNK])
                    sk = p1.tile([64, N_CHUNK], F32, tag="sk", name=f"sk{ci}")
                    nc.sync.dma_start(out=sk, in_=sink[:, off:off + N_CHUNK])

                    for m in range(2):
                        q_ps = ps1.tile([128, N_CHUNK], F32, tag="st", name=f"qps{ci}_{m}")
                        for k in range(8):
                            nc.tensor.matmul(q_ps[:],
                                             wq_sb[:, k, m * 128:(m + 1) * 128],
                                             xt[:, k, :],
                                             start=(k == 0), stop=(k == 7))
                        qtr = p1.tile([128, N_CHUNK], F32, tag="qtr", name=f"qtr{ci}_{m}")
                        nc.vector.tensor_copy(qtr[:], q_ps[:])
                        sqq = p1.tile([128, N_CHUNK], F32R, tag="sqq", name=f"sqq{ci}_{m}")
                        nc.vector.tensor_mul(sqq[:], qtr[:], qtr[:])
                        nrm_ps = ps1b.tile([2, N_CHUNK], F32, tag="st", name=f"nrm{ci}_{m}")
                        nc.tensor.matmul(nrm_ps[:], onesq[:], sqq[:],
                                         start=True, stop=True)
                        nsb = p1.tile([2, N_CHUNK], F32, tag="nsb", name=f"nsb{ci}_{m}")
                        nc.scalar.activation(out=nsb[:], in_=nrm_ps[:],
                                             func=mybir.ActivationFunctionType.Sqrt,
                                             bias=eps2[:], scale=1.0 / HD)
                        nsb2 = p1.tile([2, N_CHUNK], F32, tag="nsb2", name=f"nsb2{ci}_{m}")
                        nc.vector.reciprocal_approx_fast(out=nsb2[:], in_=nsb[:])
                        nc.vector.tensor_copy(nrq[m][:, off:off + N_CHUNK], nsb2[:])
                        # rope (pre-norm): t1a = qtr*cos + swap(qtr)*sin
                        qrot = p1.tile([128, N_CHUNK], F32, tag="qrot", name=f"qrot{ci}_{m}")
                        for blk, src in enumerate((32, 0, 96, 64)):
                            nc.sync.dma_start(out=qrot[blk * 32:(blk + 1) * 32, :],
                                              in_=qtr[src:src + 32, :])
                        tq = p1.tile([128, N_CHUNK], F32, tag="tq", name=f"tq{ci}_{m}")
                        nc.vector.tensor_mul(tq[:], qtr[:], cq[:])
                        nc.vector.tensor_mul(qrot[:], qrot[:], sq[:])
                        nc.vector.tensor_add(
                            t1a[m][:, off:off + N_CHUNK].bitcast(F32R),
                            tq[:], qrot[:])

                    kv_ps = ps1.tile([128, N_CHUNK], F32, tag="st", name=f"kvps{ci}")
                    for k in range(8):
                        nc.tensor.matmul(kv_ps[:], wkv_sb[:, k, :], xt[:, k, :],
                                         start=(k == 0), stop=(k == 7))
                    ktr = p1.tile([64, N_CHUNK], F32, tag="ktr", name=f"ktr{ci}")
                    nc.vector.tensor_copy(ktr[:], kv_ps[0:64, :])
                    vtr = p1.tile([64, N_CHUNK], F32, tag="vtr", name=f"vtr{ci}")
                    nc.vector.tensor_copy(vtr[:], kv_ps[64:128, :])
                    sqk = p1.tile([64, N_CHUNK], F32R, tag="sqk", name=f"sqk{ci}")
                    nc.vector.tensor_mul(sqk[:], ktr[:], ktr[:])
                    nkT_ps = ps1b.tile([128, 8], F32, tag="st", name=f"nkT{ci}")
                    for t in range(4):
                        nc.tensor.matmul(nkT_ps[:, 2 * t:2 * t + 2],
                                         sqk[:, t * 128:(t + 1) * 128], onesk[:],
                                         start=(t == 0), stop=(t == 3))
                    rkS = p1.tile([128, 8], F32, tag="rkS", name=f"rkS{ci}")
                    nc.scalar.activation(out=rkS[:], in_=nkT_ps[:],
                                         func=mybir.ActivationFunctionType.Sqrt,
                                         bias=eps128[:], scale=1.0 / HD)
                    nc.vector.reciprocal_approx_fast(out=rkT[:, 8 * ci:8 * ci + 8],
                                                     in_=rkS[:])
                    krot = p1.tile([64, N_CHUNK], F32, tag="krot", name=f"krot{ci}")
                    nc.sync.dma_start(out=krot[0:32, :], in_=ktr[32:64, :])
                    nc.sync.dma_start(out=krot[32:64, :], in_=ktr[0:32, :])
                    k1 = p1.tile([64, N_CHUNK], F32, tag="k1", name=f"k1{ci}")
                    nc.vector.tensor_mul(k1[:], ktr[:], ck[:])
                    nc.vector.tensor_mul(krot[:], krot[:], sk[:])
                    nc.vector.tensor_add(kt2[0:64, off:off + N_CHUNK], k1[:], krot[:])
                    nc.sync.dma_start(out=kt2[64:128, off:off + N_CHUNK],
                                      in_=kt2[0:64, off:off + N_CHUNK])
                    for t in range(4):
                        j = 4 * ci + t
                        tr_ps = ps1b.tile([128, 64], F32, tag="st", name=f"tr{ci}_{t}")
                        nc.tensor.transpose(tr_ps[:], vtr[:, t * 128:(t + 1) * 128],
                                            ident[:])
                        nc.vector.tensor_copy(v_aug[:, j, 0:64], tr_ps[:])

                # ======== interleaved: norm-apply + attention + outproj
                def phase1b(ci):
                    off = ci * N_CHUNK
                    for m in range(2):
                        rep_ps = pst.tile([128, N_CHUNK], F32, tag="st",
                                          name=f"repps{ci}_{m}")
                        nc.tensor.matmul(rep_ps[:], blockind[:],
                                         nrq[m][:, off:off + N_CHUNK],
                                         start=True, stop=True)
                        nc.vector.tensor_mul(
                            t1a[m][:, off:off + N_CHUNK].bitcast(F32R),
                            t1a[m][:, off:off + N_CHUNK], rep_ps[:])

                def phase2(m, ci):
                    off = ci * N_CHUNK
                    entries = sched[ci]
                    attn_c = p2s.tile([128, N_CHUNK], BF16, tag=f"attn{m}",
                                      name=f"attn{m}_{ci}")
                    pv = [psv.tile([65, N_CHUNK], F32, tag="pv", name=f"pv{m}_{ci}_{hh}")
                          for hh in range(2)]
                    for idx_e, (j, s0, s1, mults) in enumerate(entries):
                        koff = j * 128
                        a, b_ = s0 * 128, s1 * 128
                        st = pst.tile([128, 2, N_CHUNK], F32, tag="st",
                                      name=f"st{m}_{ci}_{j}")
                        nc.tensor.matmul(
                            st[:, 0, a:b_],
                            kt2[0:64, koff:koff + 128],
                            t1a[m][0:64, off + a:off + b_].bitcast(F32R),
                            start=True, stop=True)
                        nc.tensor.matmul(
                            st[:, 1, a:b_],
                            kt2[64:128, koff:koff + 128],
                            t1a[m][64:128, off + a:off + b_].bitcast(F32R),
                            start=True, stop=True, tile_position=(64, 0))
                        pt = p2.tile([128, 2, N_CHUNK], BF16, tag="pt",
                                     name=f"pt{m}_{ci}_{j}")
                        nc.scalar.activation(
                            out=pt[:, :, a:b_], in_=st[:, :, a:b_],
                            func=mybir.ActivationFunctionType.Exp,
                            bias=bias_c[:], scale=rkT[:, 2 * j:2 * j + 1])
                        for s_, mt in mults:
                            for hh in range(2):
                                nc.vector.tensor_mul(
                                    pt[:, hh, s_ * 128:(s_ + 1) * 128],
                                    pt[:, hh, s_ * 128:(s_ + 1) * 128],
                                    masks_sb[:, mt, :])
                        first = (idx_e == 0)
                        last = (idx_e == len(entries) - 1)
                        for hh in range(2):
                            nc.tensor.matmul(pv[hh][:, a:b_],
                                             v_aug[:, j, :],
                                             pt[:, hh, a:b_],
                                             start=first, stop=last)
                    dsb = p2s.tile([1, 2, N_CHUNK], F32, tag="dsb", name=f"dsb{m}_{ci}")
                    nc.vector.tensor_copy(dsb[:, 0, :], pv[0][64:65, :])
                    nc.vector.tensor_copy(dsb[:, 1, :], pv[1][64:65, :])
                    rd = p2s.tile([1, 2, N_CHUNK], F32, tag="rd", name=f"rd{m}_{ci}")
                    nc.vector.reciprocal_approx_fast(out=rd[:], in_=dsb[:])
                    bcd = p2s.tile([64, 2, N_CHUNK], F32, tag="bcd", bufs=1,
                                   name=f"bcd{m}_{ci}")
                    nc.gpsimd.partition_broadcast(bcd[:], rd[:], channels=64)
                    for hh in range(2):
                        nc.vector.tensor_mul(
                            attn_c[hh * 64:(hh + 1) * 64, :],
                            pv[hh][0:64, :], bcd[:, hh, :])
                    return attn_c

                def phase3(ci, attn_ts):
                    off = ci * N_CHUNK
                    for mo in range(8):
                        o_ps = pst.tile([128, N_CHUNK], F32, tag="st",
                                        name=f"ops{ci}_{mo}")
                        for k2_ in range(2):
                            nc.tensor.matmul(o_ps[:],
                                             wo_sb[:, k2_, mo * 128:(mo + 1) * 128],
                                             attn_ts[k2_][:],
                                             start=(k2_ == 0), stop=(k2_ == 1))
                        o_sb = p3.tile([128, N_CHUNK], F32, tag="osb",
                                       name=f"osb{ci}_{mo}")
                        nc.vector.tensor_copy(o_sb[:], o_ps[:])
                        nc.scalar.dma_start(
                            out=outT[mo * 128:(mo + 1) * 128, off:off + N_CHUNK],
                            in_=o_sb[:])

                for ci in range(N_CHUNKS):
                    phase1a(ci)
                    phase1b(ci)
                    a0 = phase2(0, ci)
                    a1 = phase2(1, ci)
                    phase3(ci, (a0, a1))

    nc.compile()
    return nc


def _get_nc(sched_key, sched, n_masks, neg_c):
    key = (sched_key, n_masks, float(neg_c))
    if key not in _BUILD_CACHE:
        _BUILD_CACHE[key] = _build(sched_key, sched, n_masks, neg_c)
    return _BUILD_CACHE[key]


def kernel(x, Wq, Wkv, Wo, q_norm_w, k_norm_w, rope_cos, rope_sin,
           attention_mask):
    x = np.asarray(x, dtype=np.float32)
    Wq = np.asarray(Wq, dtype=np.float32)
    Wkv = np.asarray(Wkv, dtype=np.float32)
    Wo = np.asarray(Wo, dtype=np.float32)
    qw = np.asarray(q_norm_w, dtype=np.float32)
    kw = np.asarray(k_norm_w, dtype=np.float32)
    cos = np.asarray(rope_cos, dtype=np.float32)
    sin = np.asarray(rope_sin, dtype=np.float32)

    status, mask_tiles, idx = _analyze_mask(attention_mask)
    sched = _make_schedule(status, idx)
    sched_key = status.tobytes()

    # numerically safe exp shift (0 in the normal regime)
    mct_q = max(np.abs(cos).max(), np.abs(sin).max(), 1e-9)
    bound = SCALE * 2.0 * HD * mct_q * mct_q \
        * max(np.abs(qw).max(), 1e-9) * max(np.abs(kw).max(), 1e-9)
    neg_c = -max(0.0, float(bound) - 60.0)

    nc = _get_nc(sched_key, sched, mask_tiles.shape[0], neg_c)

    # host-folded rope tables (transposed layout, head-dim on partitions)
    half = HD // 2
    swap = np.concatenate([np.arange(half, HD), np.arange(0, half)])
    sgn = np.concatenate([-np.ones(half, np.float32), np.ones(half, np.float32)])
    cosq_h = (cos.T * qw[:, None] * SCALE).astype(np.float32)          # (64, S)
    sinq_h = (sin.T * (sgn * qw[swap])[:, None] * SCALE).astype(np.float32)
    cosk_h = (cos.T * kw[:, None]).astype(np.float32)
    sink_h = (sin.T * (sgn * kw[swap])[:, None]).astype(np.float32)
    cosq2 = np.ascontiguousarray(np.concatenate([cosq_h, cosq_h], axis=0))
    sinq2 = np.ascontiguousarray(np.concatenate([sinq_h, sinq_h], axis=0))

    in_maps = []
    for c in range(8):
        b, g = c // 4, c % 4
        im = {
            "xT": np.ascontiguousarray(x[b].T),
            "wq": np.ascontiguousarray(Wq[:, g * 256:(g + 1) * 256]),
            "wkv": np.ascontiguousarray(
                np.concatenate([Wkv[:, g * HD:(g + 1) * HD],
                                Wkv[:, KVH * HD + g * HD: KVH * HD + (g + 1) * HD]],
                               axis=1)),
            "wo": np.ascontiguousarray(Wo[g * 256:(g + 1) * 256, :]).astype(ml_dtypes.bfloat16),
            "cosq": cosq2, "sinq": sinq2,
            "cosk": np.ascontiguousarray(cosk_h),
            "sink": np.ascontiguousarray(sink_h),
            "masks": mask_tiles.astype(ml_dtypes.bfloat16),
            "blockind": _BLOCKIND,
        }
        in_maps.append(im)

    from concourse.bass_utils import run_bass_kernel_spmd
    res = run_bass_kernel_spmd(nc, in_maps, core_ids=list(range(8)), trace=False)

    out = np.zeros((B, S, DIM), dtype=np.float32)
    for c in range(8):
        out[c // 4] += res.results[c]["outT"].T
    return out



# revision 21
# speedup vs baseline: 1.1883x; 1.1883x over previous
"""Block-causal GQA attention for Trainium2, 8 NeuronCores.

Sharding: core = (batch b, GQA group g): 2 batches x 4 kv-groups.
Each core computes its 4 q-heads + 1 kv-head on one batch element in a
"transposed" layout (head_dim on partitions, tokens on free dim), then a
row-parallel partial out-projection; the host sums the 4 partials per batch.

v2 layout/engine notes:
- Whole x resident in SBUF (64KB/partition), host-packed so every initial
  DMA is 128 contiguous per-partition descriptors.
- Scores run in bf16 (q~/k~ tiles bf16): 1 cy/row at any free size, keeps
  the PE HAM p-state fed; projections stay float32r.
- Head-dim stored pair-interleaved (perm[2j]=j, perm[2j+1]=j+32) so
  rotate_half becomes an adjacent-partition swap: one DVE stream_shuffle.
- All rsqrt/recip for RMS norms via scalar Ln->Exp (the natural_log_exp
  activation table also serves Exp/Square/Copy: zero table reloads).
- V transposed into (token, d) layout by the DMA xbar transpose engine.
- rope adds + mask multiplies on GpSimd (Pool); PSUM evacuation split
  DVE/scalar; denominator comes free as a 65th ones-row on V.
- Out-projection of chunk ci-1 is interleaved into the attention loop of
  chunk ci so PE/scalar never idle at phase boundaries.
"""
import numpy as np
import ml_dtypes

B, S, DIM = 2, 2048, 1024
H, KVH, HD = 16, 4, 64
EPS = 1e-6
SCALE = HD ** -0.5
PT_TILES = S // 128  # 16
N_CHUNK = 512
N_CHUNKS = S // N_CHUNK  # 4

_BUILD_CACHE = {}
_DEBUG = False
_BLOCKIND = np.zeros((2, 128), np.float32)
_BLOCKIND[0, 0:64] = 1.0
_BLOCKIND[1, 64:128] = 1.0

# pair-interleaved head-dim permutation: position 2j <- d j, 2j+1 <- d j+32
_PERM = np.empty(64, np.int64)
_PERM[0::2] = np.arange(32)
_PERM[1::2] = np.arange(32, 64)
_PERM_SW = _PERM[np.arange(64) ^ 1]          # partner (orig idx) per position
_SGN = np.where(np.arange(64) % 2 == 0, -1.0, 1.0).astype(np.float32)
_SHUF_MASK = [i ^ 1 for i in range(32)]


def _analyze_mask(mask):
    """Classify 128x128 tiles: 0=skip, 1=full, 2=mixed. Returns status grid,
    mixed tile stack (transposed to (k,q) layout, 0/1 float32), and index map.
    Index 0 of the stack is always the all-zero tile."""
    T = PT_TILES
    status = np.zeros((T, T), np.int8)
    tiles = [np.zeros((128, 128), np.float32)]
    idx = {}
    m = np.asarray(mask)
    for i in range(T):
        for j in range(T):
            sub = m[i * 128:(i + 1) * 128, j * 128:(j + 1) * 128]
            if not sub.any():
                status[i, j] = 0
            elif sub.all():
                status[i, j] = 1
            else:
                status[i, j] = 2
                idx[(i, j)] = len(tiles)
                tiles.append(np.ascontiguousarray(sub.T).astype(np.float32))
    return status, np.stack(tiles), idx


def _make_schedule(status, idx):
    """Per chunk: list of (ktile j, s0, s1, [(subtile s, mask_tile_index)])."""
    sched = []
    for ci in range(N_CHUNKS):
        qts = list(range(4 * ci, 4 * ci + 4))
        entries = []
        for j in range(PT_TILES):
            st = [status[i, j] for i in qts]
            if not any(st):
                continue
            alive = [s for s in range(4) if st[s] != 0]
            s0, s1 = alive[0], alive[-1] + 1
            mults = []
            for s in range(s0, s1):
                if st[s] == 1:
                    continue
                mults.append((s, 0 if st[s] == 0 else idx[(qts[s], j)]))
            entries.append((j, s0, s1, mults))
        sched.append(entries)
    return sched


def _build(sched_key, sched, n_masks, neg_c):
    import concourse.bacc as bacc
    import concourse.mybir as mybir
    import concourse.tile as tile

    F32 = mybir.dt.float32
    F32R = mybir.dt.float32r
    BF16 = mybir.dt.bfloat16
    LN = mybir.ActivationFunctionType.Ln
    EXPF = mybir.ActivationFunctionType.Exp

    nc = bacc.Bacc("TRN2", target_bir_lowering=False, debug=False)
    # host-packed dram tensors: every load is contiguous per partition
    xp = nc.dram_tensor("xp", (128, N_CHUNKS, 8, N_CHUNK), F32R,
                        kind="ExternalInput").ap()
    wq = nc.dram_tensor("wq", (128, 8, 256), F32R, kind="ExternalInput").ap()
    wkv = nc.dram_tensor("wkv", (128, 8, 128), F32R, kind="ExternalInput").ap()
    wo = nc.dram_tensor("wo", (128, 2, DIM), BF16, kind="ExternalInput").ap()
    tab = nc.dram_tensor("tab", (128, 4, S), BF16, kind="ExternalInput").ap()
    masks = nc.dram_tensor("masks", (128, n_masks, 128), BF16,
                           kind="ExternalInput").ap()
    blockind_d = nc.dram_tensor("blockind", (2, 128), F32R,
                                kind="ExternalInput").ap()
    outT = nc.dram_tensor("outT", (DIM, S), F32, kind="ExternalOutput").ap()
    if _DEBUG:
        dbg_t1a0 = nc.dram_tensor("dbg_t1a0", (128, S), BF16,
                                  kind="ExternalOutput").ap()
        dbg_kt2 = nc.dram_tensor("dbg_kt2", (128, S), BF16,
                                 kind="ExternalOutput").ap()
        dbg_vaug = nc.dram_tensor("dbg_vaug", (128, PT_TILES * 80), BF16,
                                  kind="ExternalOutput").ap()
        dbg_rkT = nc.dram_tensor("dbg_rkT", (128, 2 * PT_TILES), F32,
                                 kind="ExternalOutput").ap()
        dbg_attn = nc.dram_tensor("dbg_attn", (2, 128, N_CHUNK), BF16,
                                  kind="ExternalOutput").ap()
        dbg_pv = nc.dram_tensor("dbg_pv", (65, 2 * N_CHUNK), F32,
                                kind="ExternalOutput").ap()

    with tile.TileContext(nc) as tc:
        with tc.tile_pool(name="persist", bufs=1) as pp:
            # --- persistent tiles; DMA order = need order ----------------
            wq_sb = pp.tile([128, 8, 256], F32R)
            nc.sync.dma_start(out=wq_sb, in_=wq)
            xp_sb = pp.tile([128, N_CHUNKS, 8, N_CHUNK], F32R)
            nc.sync.dma_start(out=xp_sb[:, 0, 0:4], in_=xp[:, 0, 0:4])
            nc.sync.dma_start(out=xp_sb[:, 0, 4:8], in_=xp[:, 0, 4:8])
            tab_sb = pp.tile([128, 4, S], BF16)
            for ti in range(2):
                nc.sync.dma_start(out=tab_sb[:, ti], in_=tab[:, ti])
            nc.sync.dma_start(out=tab_sb[0:64, 2:4], in_=tab[0:64, 2:4])
            wkv_sb = pp.tile([128, 8, 128], F32R)
            nc.sync.dma_start(out=wkv_sb, in_=wkv)
            wo_sb = pp.tile([128, 2, DIM], BF16)
            nc.sync.dma_start(out=wo_sb, in_=wo)
            masks_sb = pp.tile([128, n_masks, 128], BF16)
            nc.sync.dma_start(out=masks_sb, in_=masks)
            blockind = pp.tile([2, 128], F32R)
            nc.sync.dma_start(out=blockind[:], in_=blockind_d)
            for ci in range(1, N_CHUNKS):
                nc.sync.dma_start(out=xp_sb[:, ci, 0:4], in_=xp[:, ci, 0:4])
                nc.sync.dma_start(out=xp_sb[:, ci, 4:8], in_=xp[:, ci, 4:8])

            t1a = [pp.tile([128, S], BF16, tag=f"t1a{m}", name=f"t1a{m}")
                   for m in range(2)]
            kt2 = pp.tile([128, S], BF16)
            v_aug = pp.tile([128, PT_TILES, 80], BF16)
            rkT = pp.tile([128, 2 * PT_TILES], F32)
            nrq = [pp.tile([2, S], F32, tag=f"nrq{m}", name=f"nrq{m}")
                   for m in range(2)]

            nc.vector.memset(v_aug[:, :, 64:65], 1.0)
            oq_f = pp.tile([128, 2], BF16)
            nc.vector.memset(oq_f, 0.0)
            nc.vector.memset(oq_f[0:64, 0:1], 1.0)
            nc.vector.memset(oq_f[64:128, 1:2], 1.0)
            ok_f = pp.tile([64, 2], BF16)
            nc.vector.memset(ok_f, 1.0)
            eps2 = pp.tile([2, 1], F32)
            nc.vector.memset(eps2, EPS)
            eps128 = pp.tile([128, 1], F32)
            nc.vector.memset(eps128, EPS)
            bias_c = pp.tile([128, 1], F32)
            nc.vector.memset(bias_c, neg_c)

            with tc.tile_pool(name="p1", bufs=2) as p1, \
                 tc.tile_pool(name="p2", bufs=3) as p2, \
                 tc.tile_pool(name="p2s", bufs=2) as p2s, \
                 tc.tile_pool(name="p3", bufs=3) as p3, \
                 tc.tile_pool(name="pst", bufs=2, space="PSUM") as pst, \
                 tc.tile_pool(name="ppv", bufs=1, space="PSUM") as ppv, \
                 tc.tile_pool(name="pbig", bufs=2, space="PSUM") as pbig:

                def phase1a(ci):
                    off = ci * N_CHUNK
                    xt = xp_sb[:, ci]
                    nrm2 = pst.tile([2, 2, N_CHUNK], F32, tag="st",
                                    name=f"nrm2_{ci}")
                    for m in range(2):
                        q_ps = pbig.tile([128, N_CHUNK], F32, tag="big",
                                         name=f"qps{ci}_{m}")
                        for k in range(8):
                            nc.tensor.matmul(q_ps[:],
                                             wq_sb[:, k, m * 128:(m + 1) * 128],
                                             xt[:, k, :],
                                             start=(k == 0), stop=(k == 7))
                        qtr = p1.tile([128, N_CHUNK], BF16, tag="qtr",
                                      name=f"qtr{ci}_{m}")
                        nc.vector.tensor_copy(qtr[:], q_ps[:])
                        sqq = p1.tile([128, N_CHUNK], BF16, tag="sqq",
                                      name=f"sqq{ci}_{m}")
                        nc.scalar.square(sqq[:], q_ps[:])
                        nc.tensor.matmul(nrm2[:, m, :], oq_f[:], sqq[:],
                                         start=True, stop=True)
                        qrot = p1.tile([128, N_CHUNK], BF16, tag="qrot",
                                       name=f"qrot{ci}_{m}")
                        nc.vector.stream_shuffle(qrot[:], qtr[:], _SHUF_MASK)
                        tq = p1.tile([128, N_CHUNK], BF16, tag="tq",
                                     name=f"tq{ci}_{m}")
                        nc.vector.tensor_mul(tq[:], qtr[:],
                                             tab_sb[:, 0, off:off + N_CHUNK])
                        nc.vector.tensor_mul(qrot[:], qrot[:],
                                             tab_sb[:, 1, off:off + N_CHUNK])
                        nc.gpsimd.tensor_add(t1a[m][:, off:off + N_CHUNK],
                                             tq[:], qrot[:])
                    nln = p1.tile([2, 2, N_CHUNK], F32, tag="nln",
                                  name=f"nln{ci}")
                    nc.scalar.activation(out=nln[:], in_=nrm2[:],
                                         func=LN, bias=eps2[:], scale=1.0 / HD)
                    nrq2 = p1.tile([2, 2, N_CHUNK], F32R, tag="nrq2",
                                   name=f"nrq2_{ci}")
                    nc.scalar.activation(out=nrq2[:], in_=nln[:],
                                         func=EXPF, scale=-0.5)

                    kv_ps = pbig.tile([128, N_CHUNK], F32, tag="big",
                                      name=f"kvps{ci}")
                    for k in range(8):
                        nc.tensor.matmul(kv_ps[:], wkv_sb[:, k, :], xt[:, k, :],
                                         start=(k == 0), stop=(k == 7))
                    ktr = p1.tile([64, N_CHUNK], BF16, tag="ktr",
                                  name=f"ktr{ci}")
                    nc.vector.tensor_copy(ktr[:], kv_ps[0:64, :])
                    vtr = p1.tile([64, N_CHUNK], BF16, tag="vtr",
                                  name=f"vtr{ci}")
                    nc.vector.tensor_copy(vtr[:], kv_ps[64:128, :])
                    sqk = p1.tile([64, N_CHUNK], BF16, tag="sqk",
                                  name=f"sqk{ci}")
                    nc.scalar.square(sqk[:], kv_ps[0:64, :])
                    nkT_ps = pbig.tile([128, 8], F32, tag="big",
                                       name=f"nkT{ci}")
                    for t in range(4):
                        nc.tensor.matmul(nkT_ps[:, 2 * t:2 * t + 2],
                                         sqk[:, t * 128:(t + 1) * 128],
                                         ok_f[:],
                                         start=(t == 0), stop=(t == 3))
                    rkS = p1.tile([128, 8], F32, tag="rkS", name=f"rkS{ci}")
                    nc.scalar.activation(out=rkS[:], in_=nkT_ps[:],
                                         func=LN, bias=eps128[:], scale=1.0 / HD)
                    nc.scalar.activation(out=rkT[:, 8 * ci:8 * ci + 8],
                                         in_=rkS[:], func=EXPF, scale=-0.5)
                    krot = p1.tile([64, N_CHUNK], BF16, tag="krot",
                                   name=f"krot{ci}")
                    nc.vector.stream_shuffle(krot[:], ktr[:], _SHUF_MASK)
                    k1 = p1.tile([64, N_CHUNK], BF16, tag="k1", name=f"k1{ci}")
                    nc.vector.tensor_mul(k1[:], ktr[:],
                                         tab_sb[0:64, 2, off:off + N_CHUNK])
                    nc.vector.tensor_mul(krot[:], krot[:],
                                         tab_sb[0:64, 3, off:off + N_CHUNK])
                    nc.gpsimd.tensor_add(kt2[0:64, off:off + N_CHUNK],
                                         k1[:], krot[:])
                    nc.sync.dma_start(out=kt2[64:128, off:off + N_CHUNK],
                                      in_=kt2[0:64, off:off + N_CHUNK])
                    for t in range(4):
                        nc.sync.dma_start_transpose(
                            v_aug[:, 4 * ci + t, 0:64],
                            vtr[:, t * 128:(t + 1) * 128])
                    return nrq2

                def phase1b(ci, nrq2):
                    off = ci * N_CHUNK
                    for m in range(2):
                        rep_ps = pbig.tile([128, N_CHUNK], F32, tag="big",
                                           name=f"repps{ci}_{m}")
                        nc.tensor.matmul(rep_ps[:], blockind[:],
                                         nrq2[:, m, :],
                                         start=True, stop=True)
                        nc.vector.tensor_mul(
                            t1a[m][:, off:off + N_CHUNK],
                            t1a[m][:, off:off + N_CHUNK], rep_ps[:])

                def phase3_mo(ci, attn_ts, mo):
                    off = ci * N_CHUNK
                    o_ps = pbig.tile([128, N_CHUNK], F32, tag="big",
                                     name=f"ops{ci}_{mo}")
                    for k2_ in range(2):
                        nc.tensor.matmul(o_ps[:],
                                         wo_sb[:, k2_, mo * 128:(mo + 1) * 128],
                                         attn_ts[k2_][:],
                                         start=(k2_ == 0), stop=(k2_ == 1))
                    o_sb = p3.tile([128, N_CHUNK], F32, tag="osb",
                                   name=f"osb{ci}_{mo}")
                    nc.vector.tensor_copy(o_sb[:], o_ps[:])
                    nc.scalar.dma_start(
                        out=outT[mo * 128:(mo + 1) * 128, off:off + N_CHUNK],
                        in_=o_sb[:])

                def phase2(m, ci, deferred):
                    off = ci * N_CHUNK
                    entries = sched[ci]
                    attn_c = p2s.tile([128, N_CHUNK], BF16, tag=f"attn{m}",
                                      name=f"attn{m}_{ci}")
                    pv = ppv.tile([65, 2, N_CHUNK], F32, tag="pv",
                                  name=f"pv{m}_{ci}")
                    for idx_e, (j, s0, s1, mults) in enumerate(entries):
                        koff = j * 128
                        a, b_ = s0 * 128, s1 * 128
                        st = pst.tile([128, 2, N_CHUNK], F32, tag="st",
                                      name=f"st{m}_{ci}_{j}")
                        nc.tensor.matmul(
                            st[:, 0, a:b_],
                            kt2[0:64, koff:koff + 128],
                            t1a[m][0:64, off + a:off + b_],
                            start=True, stop=True)
                        nc.tensor.matmul(
                            st[:, 1, a:b_],
                            kt2[64:128, koff:koff + 128],
                            t1a[m][64:128, off + a:off + b_],
                            start=True, stop=True, tile_position=(64, 0))
                        pt = p2.tile([128, 2, N_CHUNK], BF16, tag="pt",
                                     name=f"pt{m}_{ci}_{j}")
                        nc.scalar.activation(
                            out=pt[:, :, a:b_], in_=st[:, :, a:b_],
                            func=EXPF,
                            bias=bias_c[:], scale=rkT[:, 2 * j:2 * j + 1])
                        for s_, mt in mults:
                            for hh in range(2):
                                nc.gpsimd.tensor_mul(
                                    pt[:, hh, s_ * 128:(s_ + 1) * 128],
                                    pt[:, hh, s_ * 128:(s_ + 1) * 128],
                                    masks_sb[:, mt, :])
                        first = (idx_e == 0)
                        last = (idx_e == len(entries) - 1)
                        for hh in range(2):
                            nc.tensor.matmul(pv[:, hh, a:b_],
                                             v_aug[:, j, 0:65],
                                             pt[:, hh, a:b_],
                                             start=first, stop=last)
                        # interleave previous chunk's out-projection so the
                        # PE/DVE keep busy while scalar exp paces this loop
                        if idx_e >= 1 and deferred:
                            phase3_mo(*deferred.pop(0))
                    dsb = p2s.tile([1, 2, N_CHUNK], F32, tag="dsb",
                                   name=f"dsb{m}_{ci}")
                    nc.vector.tensor_copy(dsb[:], pv[64:65, :, :])
                    rd = p2s.tile([1, 2, N_CHUNK], F32, tag="rd",
                                  name=f"rd{m}_{ci}")
                    nc.vector.reciprocal_approx_fast(out=rd[:], in_=dsb[:])
                    bcd = p2s.tile([64, 2, N_CHUNK], F32, tag="bcd", bufs=1,
                                   name=f"bcd{m}_{ci}")
                    nc.gpsimd.partition_broadcast(bcd[:], rd[:], channels=64)
                    for hh in range(2):
                        nc.vector.tensor_mul(
                            attn_c[hh * 64:(hh + 1) * 64, :],
                            pv[0:64, hh, :], bcd[:, hh, :])
                    if _DEBUG and m == 0 and ci == 0:
                        pvs = p2s.tile([65, 2, N_CHUNK], F32, tag="dbgpv",
                                       name="dbgpv")
                        nc.vector.tensor_copy(pvs[:], pv[:])
                        nc.sync.dma_start(
                            out=dbg_pv,
                            in_=pvs.rearrange("p a b -> p (a b)"))
                    return attn_c

                prev = None
                for ci in range(N_CHUNKS):
                    nrq2 = phase1a(ci)
                    phase1b(ci, nrq2)
                    deferred = []
                    if prev is not None:
                        pci, a0p, a1p = prev
                        deferred = [(pci, (a0p, a1p), mo) for mo in range(8)]
                    a0 = phase2(0, ci, deferred)
                    a1 = phase2(1, ci, deferred)
                    for d in deferred:
                        phase3_mo(*d)
                    prev = (ci, a0, a1)
                pci, a0p, a1p = prev
                for mo in range(8):
                    phase3_mo(pci, (a0p, a1p), mo)
                if _DEBUG:
                    nc.sync.dma_start(out=dbg_attn[0], in_=a0p[:])
                    nc.sync.dma_start(out=dbg_attn[1], in_=a1p[:])
                    nc.sync.dma_start(out=dbg_t1a0, in_=t1a[0][:])
                    nc.sync.dma_start(out=dbg_kt2, in_=kt2[:])
                    nc.sync.dma_start(
                        out=dbg_vaug,
                        in_=v_aug.rearrange("p a b -> p (a b)"))
                    nc.sync.dma_start(out=dbg_rkT, in_=rkT[:])

    nc.compile()
    return nc


def _get_nc(sched_key, sched, n_masks, neg_c):
    key = (sched_key, n_masks, float(neg_c))
    if key not in _BUILD_CACHE:
        _BUILD_CACHE[key] = _build(sched_key, sched, n_masks, neg_c)
    return _BUILD_CACHE[key]


def kernel(x, Wq, Wkv, Wo, q_norm_w, k_norm_w, rope_cos, rope_sin,
           attention_mask):
    x = np.asarray(x, dtype=np.float32)
    Wq = np.asarray(Wq, dtype=np.float32)
    Wkv = np.asarray(Wkv, dtype=np.float32)
    Wo = np.asarray(Wo, dtype=np.float32)
    qw = np.asarray(q_norm_w, dtype=np.float32)
    kw = np.asarray(k_norm_w, dtype=np.float32)
    cos = np.asarray(rope_cos, dtype=np.float32)
    sin = np.asarray(rope_sin, dtype=np.float32)

    status, mask_tiles, idx = _analyze_mask(attention_mask)
    sched = _make_schedule(status, idx)
    sched_key = status.tobytes()

    # numerically safe exp shift (0 in the normal regime)
    mct_q = max(np.abs(cos).max(), np.abs(sin).max(), 1e-9)
    bound = SCALE * 2.0 * HD * mct_q * mct_q \
        * max(np.abs(qw).max(), 1e-9) * max(np.abs(kw).max(), 1e-9)
    neg_c = -max(0.0, float(bound) - 60.0)

    nc = _get_nc(sched_key, sched, mask_tiles.shape[0], neg_c)

    # host-folded rope tables in the pair-interleaved d layout
    cosq_h = (cos.T[_PERM] * (qw[_PERM] * SCALE)[:, None]).astype(np.float32)
    sinq_h = (sin.T[_PERM] * (_SGN * qw[_PERM_SW] * SCALE)[:, None]
              ).astype(np.float32)
    cosk_h = (cos.T[_PERM] * kw[_PERM][:, None]).astype(np.float32)
    sink_h = (sin.T[_PERM] * (_SGN * kw[_PERM_SW])[:, None]).astype(np.float32)
    tab = np.zeros((128, 4, S), np.float32)
    tab[0:64, 0] = cosq_h
    tab[64:128, 0] = cosq_h
    tab[0:64, 1] = sinq_h
    tab[64:128, 1] = sinq_h
    tab[0:64, 2] = cosk_h
    tab[0:64, 3] = sink_h
    tab_b = tab.astype(ml_dtypes.bfloat16)

    masks_p = np.ascontiguousarray(
        mask_tiles.transpose(1, 0, 2)).astype(ml_dtypes.bfloat16)

    in_maps = []
    for c in range(8):
        b, g = c // 4, c % 4
        wq_s = Wq[:, g * 256:(g + 1) * 256].reshape(DIM, 4, 64)[:, :, _PERM]
        wq_s = wq_s.reshape(DIM, 256)
        wk_s = Wkv[:, g * HD:(g + 1) * HD][:, _PERM]
        wv_s = Wkv[:, KVH * HD + g * HD: KVH * HD + (g + 1) * HD]
        wkv_s = np.concatenate([wk_s, wv_s], axis=1)
        wo_s = Wo[g * 256:(g + 1) * 256, :]
        im = {
            "xp": np.ascontiguousarray(
                x[b].T.reshape(8, 128, N_CHUNKS, N_CHUNK)
                .transpose(1, 2, 0, 3)),
            "wq": np.ascontiguousarray(
                wq_s.reshape(8, 128, 256).transpose(1, 0, 2)),
            "wkv": np.ascontiguousarray(
                wkv_s.reshape(8, 128, 128).transpose(1, 0, 2)),
            "wo": np.ascontiguousarray(
                wo_s.reshape(2, 128, DIM).transpose(1, 0, 2)
            ).astype(ml_dtypes.bfloat16),
            "tab": tab_b,
            "masks": masks_p,
            "blockind": _BLOCKIND,
        }
        in_maps.append(im)

    from concourse.bass_utils import run_bass_kernel_spmd
    res = run_bass_kernel_spmd(nc, in_maps, core_ids=list(range(8)), trace=False)

    out = np.zeros((B, S, DIM), dtype=np.float32)
    for c in range(8):
        out[c // 4] += res.results[c]["outT"].T
    return out


# revision 22
# speedup vs baseline: 1.1971x; 1.0074x over previous
"""Block-causal GQA attention for Trainium2, 8 NeuronCores.

Sharding: core = (batch b, GQA group g): 2 batches x 4 kv-groups.
Each core computes its 4 q-heads + 1 kv-head on one batch element in a
"transposed" layout (head_dim on partitions, tokens on free dim), then a
row-parallel partial out-projection; the host sums the 4 partials per batch.

v2 layout/engine notes:
- Whole x resident in SBUF (64KB/partition), host-packed so every initial
  DMA is 128 contiguous per-partition descriptors.
- Scores run in bf16 (q~/k~ tiles bf16): 1 cy/row at any free size, keeps
  the PE HAM p-state fed; projections stay float32r.
- Head-dim stored pair-interleaved (perm[2j]=j, perm[2j+1]=j+32) so
  rotate_half becomes an adjacent-partition swap: one DVE stream_shuffle.
- All rsqrt/recip for RMS norms via scalar Ln->Exp (the natural_log_exp
  activation table also serves Exp/Square/Copy: zero table reloads).
- V transposed into (token, d) layout by the DMA xbar transpose engine.
- rope adds + mask multiplies on GpSimd (Pool); PSUM evacuation split
  DVE/scalar; denominator comes free as a 65th ones-row on V.
- Out-projection of chunk ci-1 is interleaved into the attention loop of
  chunk ci so PE/scalar never idle at phase boundaries.
"""
import numpy as np
import ml_dtypes

B, S, DIM = 2, 2048, 1024
H, KVH, HD = 16, 4, 64
EPS = 1e-6
SCALE = HD ** -0.5
PT_TILES = S // 128  # 16
N_CHUNK = 512
N_CHUNKS = S // N_CHUNK  # 4

_BUILD_CACHE = {}
_DEBUG = False
_BLOCKIND = np.zeros((2, 128), np.float32)
_BLOCKIND[0, 0:64] = 1.0
_BLOCKIND[1, 64:128] = 1.0

# pair-interleaved head-dim permutation: position 2j <- d j, 2j+1 <- d j+32
_PERM = np.empty(64, np.int64)
_PERM[0::2] = np.arange(32)
_PERM[1::2] = np.arange(32, 64)
_PERM_SW = _PERM[np.arange(64) ^ 1]          # partner (orig idx) per position
_SGN = np.where(np.arange(64) % 2 == 0, -1.0, 1.0).astype(np.float32)
_SHUF_MASK = [i ^ 1 for i in range(32)]


def _analyze_mask(mask):
    """Classify 128x128 tiles: 0=skip, 1=full, 2=mixed. Returns status grid,
    mixed tile stack (transposed to (k,q) layout, 0/1 float32), and index map.
    Index 0 of the stack is always the all-zero tile."""
    T = PT_TILES
    status = np.zeros((T, T), np.int8)
    tiles = [np.zeros((128, 128), np.float32)]
    idx = {}
    m = np.asarray(mask)
    for i in range(T):
        for j in range(T):
            sub = m[i * 128:(i + 1) * 128, j * 128:(j + 1) * 128]
            if not sub.any():
                status[i, j] = 0
            elif sub.all():
                status[i, j] = 1
            else:
                status[i, j] = 2
                idx[(i, j)] = len(tiles)
                tiles.append(np.ascontiguousarray(sub.T).astype(np.float32))
    return status, np.stack(tiles), idx


def _make_schedule(status, idx):
    """Per chunk: list of (ktile j, s0, s1, [(subtile s, mask_tile_index)])."""
    sched = []
    for ci in range(N_CHUNKS):
        qts = list(range(4 * ci, 4 * ci + 4))
        entries = []
        for j in range(PT_TILES):
            st = [status[i, j] for i in qts]
            if not any(st):
                continue
            alive = [s for s in range(4) if st[s] != 0]
            s0, s1 = alive[0], alive[-1] + 1
            mults = []
            for s in range(s0, s1):
                if st[s] == 1:
                    continue
                mults.append((s, 0 if st[s] == 0 else idx[(qts[s], j)]))
            entries.append((j, s0, s1, mults))
        sched.append(entries)
    return sched


def _build(sched_key, sched, n_masks, neg_c):
    import concourse.bacc as bacc
    import concourse.mybir as mybir
    import concourse.tile as tile

    F32 = mybir.dt.float32
    F32R = mybir.dt.float32r
    BF16 = mybir.dt.bfloat16
    LN = mybir.ActivationFunctionType.Ln
    EXPF = mybir.ActivationFunctionType.Exp

    nc = bacc.Bacc("TRN2", target_bir_lowering=False, debug=False)
    # host-packed dram tensors: every load is contiguous per partition
    xp = nc.dram_tensor("xp", (128, N_CHUNKS, 8, N_CHUNK), F32R,
                        kind="ExternalInput").ap()
    wq = nc.dram_tensor("wq", (128, 8, 256), F32R, kind="ExternalInput").ap()
    wkv = nc.dram_tensor("wkv", (128, 8, 128), F32R, kind="ExternalInput").ap()
    wo = nc.dram_tensor("wo", (128, 2, DIM), BF16, kind="ExternalInput").ap()
    tab = nc.dram_tensor("tab", (128, 4, S), BF16, kind="ExternalInput").ap()
    masks = nc.dram_tensor("masks", (128, n_masks, 128), BF16,
                           kind="ExternalInput").ap()
    blockind_d = nc.dram_tensor("blockind", (2, 128), F32R,
                                kind="ExternalInput").ap()
    outT = nc.dram_tensor("outT", (DIM, S), F32, kind="ExternalOutput").ap()
    if _DEBUG:
        dbg_t1a0 = nc.dram_tensor("dbg_t1a0", (128, S), BF16,
                                  kind="ExternalOutput").ap()
        dbg_kt2 = nc.dram_tensor("dbg_kt2", (128, S), BF16,
                                 kind="ExternalOutput").ap()
        dbg_vaug = nc.dram_tensor("dbg_vaug", (128, PT_TILES * 80), BF16,
                                  kind="ExternalOutput").ap()
        dbg_rkT = nc.dram_tensor("dbg_rkT", (128, 2 * PT_TILES), F32,
                                 kind="ExternalOutput").ap()
        dbg_attn = nc.dram_tensor("dbg_attn", (2, 128, N_CHUNK), BF16,
                                  kind="ExternalOutput").ap()
        dbg_pv = nc.dram_tensor("dbg_pv", (65, 2 * N_CHUNK), F32,
                                kind="ExternalOutput").ap()

    with tile.TileContext(nc) as tc:
        with tc.tile_pool(name="persist", bufs=1) as pp:
            # --- persistent tiles; DMA order = need order ----------------
            wq_sb = pp.tile([128, 8, 256], F32R)
            nc.sync.dma_start(out=wq_sb, in_=wq)
            xp_sb = pp.tile([128, N_CHUNKS, 8, N_CHUNK], F32R)
            nc.sync.dma_start(out=xp_sb[:, 0, 0:4], in_=xp[:, 0, 0:4])
            nc.sync.dma_start(out=xp_sb[:, 0, 4:8], in_=xp[:, 0, 4:8])
            tab_sb = pp.tile([128, 4, S], BF16)
            for ti in range(2):
                nc.sync.dma_start(out=tab_sb[:, ti], in_=tab[:, ti])
            nc.sync.dma_start(out=tab_sb[0:64, 2:4], in_=tab[0:64, 2:4])
            wkv_sb = pp.tile([128, 8, 128], F32R)
            nc.sync.dma_start(out=wkv_sb, in_=wkv)
            wo_sb = pp.tile([128, 2, DIM], BF16)
            nc.sync.dma_start(out=wo_sb, in_=wo)
            masks_sb = pp.tile([128, n_masks, 128], BF16)
            nc.sync.dma_start(out=masks_sb, in_=masks)
            blockind = pp.tile([2, 128], F32R)
            nc.sync.dma_start(out=blockind[:], in_=blockind_d)
            for ci in range(1, N_CHUNKS):
                nc.sync.dma_start(out=xp_sb[:, ci, 0:4], in_=xp[:, ci, 0:4])
                nc.sync.dma_start(out=xp_sb[:, ci, 4:8], in_=xp[:, ci, 4:8])

            t1a = [pp.tile([128, S], BF16, tag=f"t1a{m}", name=f"t1a{m}")
                   for m in range(2)]
            kt2 = pp.tile([128, S], BF16)
            v_aug = pp.tile([128, PT_TILES, 80], BF16)
            rkT = pp.tile([128, 2 * PT_TILES], F32)
            nrq = [pp.tile([2, S], F32, tag=f"nrq{m}", name=f"nrq{m}")
                   for m in range(2)]

            nc.vector.memset(v_aug[:, :, 64:65], 1.0)
            oq_f = pp.tile([128, 2], BF16)
            nc.vector.memset(oq_f, 0.0)
            nc.vector.memset(oq_f[0:64, 0:1], 1.0)
            nc.vector.memset(oq_f[64:128, 1:2], 1.0)
            ok_f = pp.tile([64, 2], BF16)
            nc.vector.memset(ok_f, 1.0)
            eps2 = pp.tile([2, 1], F32)
            nc.vector.memset(eps2, EPS)
            eps128 = pp.tile([128, 1], F32)
            nc.vector.memset(eps128, EPS)
            bias_c = pp.tile([128, 1], F32)
            nc.vector.memset(bias_c, neg_c)
            # pin the natural_log_exp activation table (serves Exp/Ln/Square/
            # Copy) so the table-load pass never thrashes between tables
            nc.scalar.add_instruction(mybir.InstLoadActFuncSet(
                name=nc.get_next_instruction_name(), ins=[], outs=[],
                act_func_set_id=6))

            with tc.tile_pool(name="p1", bufs=2) as p1, \
                 tc.tile_pool(name="p2", bufs=3) as p2, \
                 tc.tile_pool(name="p2s", bufs=2) as p2s, \
                 tc.tile_pool(name="p3", bufs=3) as p3, \
                 tc.tile_pool(name="pst", bufs=2, space="PSUM") as pst, \
                 tc.tile_pool(name="ppv", bufs=1, space="PSUM") as ppv, \
                 tc.tile_pool(name="pbig", bufs=2, space="PSUM") as pbig:

                def phase1a(ci):
                    off = ci * N_CHUNK
                    xt = xp_sb[:, ci]
                    nrm2 = pst.tile([2, 2, N_CHUNK], F32, tag="st",
                                    name=f"nrm2_{ci}")
                    for m in range(2):
                        q_ps = pbig.tile([128, N_CHUNK], F32, tag="big",
                                         name=f"qps{ci}_{m}")
                        for k in range(8):
                            nc.tensor.matmul(q_ps[:],
                                             wq_sb[:, k, m * 128:(m + 1) * 128],
                                             xt[:, k, :],
                                             start=(k == 0), stop=(k == 7))
                        qtr = p1.tile([128, N_CHUNK], BF16, tag="qtr",
                                      name=f"qtr{ci}_{m}")
                        nc.vector.tensor_copy(qtr[:], q_ps[:])
                        sqq = p1.tile([128, N_CHUNK], BF16, tag="sqq",
                                      name=f"sqq{ci}_{m}")
                        nc.scalar.square(sqq[:], q_ps[:])
                        nc.tensor.matmul(nrm2[:, m, :], oq_f[:], sqq[:],
                                         start=True, stop=True)
                        qrot = p1.tile([128, N_CHUNK], BF16, tag="qrot",
                                       name=f"qrot{ci}_{m}")
                        nc.vector.stream_shuffle(qrot[:], qtr[:], _SHUF_MASK)
                        tq = p1.tile([128, N_CHUNK], BF16, tag="tq",
                                     name=f"tq{ci}_{m}")
                        nc.vector.tensor_mul(tq[:], qtr[:],
                                             tab_sb[:, 0, off:off + N_CHUNK])
                        nc.vector.tensor_mul(qrot[:], qrot[:],
                                             tab_sb[:, 1, off:off + N_CHUNK])
                        nc.gpsimd.tensor_add(t1a[m][:, off:off + N_CHUNK],
                                             tq[:], qrot[:])
                    nln = p1.tile([2, 2, N_CHUNK], F32, tag="nln",
                                  name=f"nln{ci}")
                    nc.scalar.activation(out=nln[:], in_=nrm2[:],
                                         func=LN, bias=eps2[:], scale=1.0 / HD)
                    nrq2 = p1.tile([2, 2, N_CHUNK], F32R, tag="nrq2",
                                   name=f"nrq2_{ci}")
                    nc.scalar.activation(out=nrq2[:], in_=nln[:],
                                         func=EXPF, scale=-0.5)

                    kv_ps = pbig.tile([128, N_CHUNK], F32, tag="big",
                                      name=f"kvps{ci}")
                    for k in range(8):
                        nc.tensor.matmul(kv_ps[:], wkv_sb[:, k, :], xt[:, k, :],
                                         start=(k == 0), stop=(k == 7))
                    ktr = p1.tile([64, N_CHUNK], BF16, tag="ktr",
                                  name=f"ktr{ci}")
                    nc.vector.tensor_copy(ktr[:], kv_ps[0:64, :])
                    vtr = p1.tile([64, N_CHUNK], BF16, tag="vtr",
                                  name=f"vtr{ci}")
                    nc.vector.tensor_copy(vtr[:], kv_ps[64:128, :])
                    sqk = p1.tile([64, N_CHUNK], BF16, tag="sqk",
                                  name=f"sqk{ci}")
                    nc.scalar.square(sqk[:], kv_ps[0:64, :])
                    nkT_ps = pbig.tile([128, 8], F32, tag="big",
                                       name=f"nkT{ci}")
                    for t in range(4):
                        nc.tensor.matmul(nkT_ps[:, 2 * t:2 * t + 2],
                                         sqk[:, t * 128:(t + 1) * 128],
                                         ok_f[:],
                                         start=(t == 0), stop=(t == 3))
                    rkS = p1.tile([128, 8], F32, tag="rkS", name=f"rkS{ci}")
                    nc.scalar.activation(out=rkS[:], in_=nkT_ps[:],
                                         func=LN, bias=eps128[:], scale=1.0 / HD)
                    nc.scalar.activation(out=rkT[:, 8 * ci:8 * ci + 8],
                                         in_=rkS[:], func=EXPF, scale=-0.5)
                    krot = p1.tile([64, N_CHUNK], BF16, tag="krot",
                                   name=f"krot{ci}")
                    nc.vector.stream_shuffle(krot[:], ktr[:], _SHUF_MASK)
                    k1 = p1.tile([64, N_CHUNK], BF16, tag="k1", name=f"k1{ci}")
                    nc.vector.tensor_mul(k1[:], ktr[:],
                                         tab_sb[0:64, 2, off:off + N_CHUNK])
                    nc.vector.tensor_mul(krot[:], krot[:],
                                         tab_sb[0:64, 3, off:off + N_CHUNK])
                    nc.gpsimd.tensor_add(kt2[0:64, off:off + N_CHUNK],
                                         k1[:], krot[:])
                    nc.sync.dma_start(out=kt2[64:128, off:off + N_CHUNK],
                                      in_=kt2[0:64, off:off + N_CHUNK])
                    for t in range(4):
                        nc.sync.dma_start_transpose(
                            v_aug[:, 4 * ci + t, 0:64],
                            vtr[:, t * 128:(t + 1) * 128])
                    return nrq2

                def phase1b(ci, nrq2):
                    off = ci * N_CHUNK
                    for m in range(2):
                        rep_ps = pbig.tile([128, N_CHUNK], F32, tag="big",
                                           name=f"repps{ci}_{m}")
                        nc.tensor.matmul(rep_ps[:], blockind[:],
                                         nrq2[:, m, :],
                                         start=True, stop=True)
                        nc.vector.tensor_mul(
                            t1a[m][:, off:off + N_CHUNK],
                            t1a[m][:, off:off + N_CHUNK], rep_ps[:])

                def phase3_mo(ci, attn_ts, mo):
                    off = ci * N_CHUNK
                    o_ps = pbig.tile([128, N_CHUNK], F32, tag="big",
                                     name=f"ops{ci}_{mo}")
                    for k2_ in range(2):
                        nc.tensor.matmul(o_ps[:],
                                         wo_sb[:, k2_, mo * 128:(mo + 1) * 128],
                                         attn_ts[k2_][:],
                                         start=(k2_ == 0), stop=(k2_ == 1))
                    o_sb = p3.tile([128, N_CHUNK], F32, tag="osb",
                                   name=f"osb{ci}_{mo}")
                    nc.vector.tensor_copy(o_sb[:], o_ps[:])
                    nc.scalar.dma_start(
                        out=outT[mo * 128:(mo + 1) * 128, off:off + N_CHUNK],
                        in_=o_sb[:])

                def phase2(m, ci, deferred):
                    off = ci * N_CHUNK
                    entries = sched[ci]
                    attn_c = p2s.tile([128, N_CHUNK], BF16, tag=f"attn{m}",
                                      name=f"attn{m}_{ci}")
                    pv = ppv.tile([65, 2, N_CHUNK], F32, tag="pv",
                                  name=f"pv{m}_{ci}")
                    for idx_e, (j, s0, s1, mults) in enumerate(entries):
                        koff = j * 128
                        a, b_ = s0 * 128, s1 * 128
                        st = pst.tile([128, 2, N_CHUNK], F32, tag="st",
                                      name=f"st{m}_{ci}_{j}")
                        nc.tensor.matmul(
                            st[:, 0, a:b_],
                            kt2[0:64, koff:koff + 128],
                            t1a[m][0:64, off + a:off + b_],
                            start=True, stop=True)
                        nc.tensor.matmul(
                            st[:, 1, a:b_],
                            kt2[64:128, koff:koff + 128],
                            t1a[m][64:128, off + a:off + b_],
                            start=True, stop=True, tile_position=(64, 0))
                        pt = p2.tile([128, 2, N_CHUNK], BF16, tag="pt",
                                     name=f"pt{m}_{ci}_{j}")
                        nc.scalar.activation(
                            out=pt[:, :, a:b_], in_=st[:, :, a:b_],
                            func=EXPF,
                            bias=bias_c[:], scale=rkT[:, 2 * j:2 * j + 1])
                        for s_, mt in mults:
                            for hh in range(2):
                                nc.gpsimd.tensor_mul(
                                    pt[:, hh, s_ * 128:(s_ + 1) * 128],
                                    pt[:, hh, s_ * 128:(s_ + 1) * 128],
                                    masks_sb[:, mt, :])
                        first = (idx_e == 0)
                        last = (idx_e == len(entries) - 1)
                        for hh in range(2):
                            nc.tensor.matmul(pv[:, hh, a:b_],
                                             v_aug[:, j, 0:65],
                                             pt[:, hh, a:b_],
                                             start=first, stop=last)
                        # interleave previous chunk's out-projection so the
                        # PE/DVE keep busy while scalar exp paces this loop
                        if idx_e >= 1 and deferred:
                            phase3_mo(*deferred.pop(0))
                    dsb = p2s.tile([1, 2, N_CHUNK], F32, tag="dsb",
                                   name=f"dsb{m}_{ci}")
                    nc.vector.tensor_copy(dsb[:], pv[64:65, :, :])
                    rd = p2s.tile([1, 2, N_CHUNK], F32, tag="rd",
                                  name=f"rd{m}_{ci}")
                    nc.vector.reciprocal_approx_fast(out=rd[:], in_=dsb[:])
                    bcd = p2s.tile([64, 2, N_CHUNK], F32, tag="bcd", bufs=1,
                                   name=f"bcd{m}_{ci}")
                    nc.gpsimd.partition_broadcast(bcd[:], rd[:], channels=64)
                    for hh in range(2):
                        nc.vector.tensor_mul(
                            attn_c[hh * 64:(hh + 1) * 64, :],
                            pv[0:64, hh, :], bcd[:, hh, :])
                    if _DEBUG and m == 0 and ci == 0:
                        pvs = p2s.tile([65, 2, N_CHUNK], F32, tag="dbgpv",
                                       name="dbgpv")
                        nc.vector.tensor_copy(pvs[:], pv[:])
                        nc.sync.dma_start(
                            out=dbg_pv,
                            in_=pvs.rearrange("p a b -> p (a b)"))
                    return attn_c

                prev = None
                for ci in range(N_CHUNKS):
                    nrq2 = phase1a(ci)
                    phase1b(ci, nrq2)
                    deferred = []
                    if prev is not None:
                        pci, a0p, a1p = prev
                        deferred = [(pci, (a0p, a1p), mo) for mo in range(8)]
                    a0 = phase2(0, ci, deferred)
                    a1 = phase2(1, ci, deferred)
                    for d in deferred:
                        phase3_mo(*d)
                    prev = (ci, a0, a1)
                pci, a0p, a1p = prev
                for mo in range(8):
                    phase3_mo(pci, (a0p, a1p), mo)
                if _DEBUG:
                    nc.sync.dma_start(out=dbg_attn[0], in_=a0p[:])
                    nc.sync.dma_start(out=dbg_attn[1], in_=a1p[:])
                    nc.sync.dma_start(out=dbg_t1a0, in_=t1a[0][:])
                    nc.sync.dma_start(out=dbg_kt2, in_=kt2[:])
                    nc.sync.dma_start(
                        out=dbg_vaug,
                        in_=v_aug.rearrange("p a b -> p (a b)"))
                    nc.sync.dma_start(out=dbg_rkT, in_=rkT[:])

    nc.compile()
    return nc


def _get_nc(sched_key, sched, n_masks, neg_c):
    key = (sched_key, n_masks, float(neg_c))
    if key not in _BUILD_CACHE:
        _BUILD_CACHE[key] = _build(sched_key, sched, n_masks, neg_c)
    return _BUILD_CACHE[key]


def kernel(x, Wq, Wkv, Wo, q_norm_w, k_norm_w, rope_cos, rope_sin,
           attention_mask):
    x = np.asarray(x, dtype=np.float32)
    Wq = np.asarray(Wq, dtype=np.float32)
    Wkv = np.asarray(Wkv, dtype=np.float32)
    Wo = np.asarray(Wo, dtype=np.float32)
    qw = np.asarray(q_norm_w, dtype=np.float32)
    kw = np.asarray(k_norm_w, dtype=np.float32)
    cos = np.asarray(rope_cos, dtype=np.float32)
    sin = np.asarray(rope_sin, dtype=np.float32)

    status, mask_tiles, idx = _analyze_mask(attention_mask)
    sched = _make_schedule(status, idx)
    sched_key = status.tobytes()

    # numerically safe exp shift (0 in the normal regime)
    mct_q = max(np.abs(cos).max(), np.abs(sin).max(), 1e-9)
    bound = SCALE * 2.0 * HD * mct_q * mct_q \
        * max(np.abs(qw).max(), 1e-9) * max(np.abs(kw).max(), 1e-9)
    neg_c = -max(0.0, float(bound) - 60.0)

    nc = _get_nc(sched_key, sched, mask_tiles.shape[0], neg_c)

    # host-folded rope tables in the pair-interleaved d layout
    cosq_h = (cos.T[_PERM] * (qw[_PERM] * SCALE)[:, None]).astype(np.float32)
    sinq_h = (sin.T[_PERM] * (_SGN * qw[_PERM_SW] * SCALE)[:, None]
              ).astype(np.float32)
    cosk_h = (cos.T[_PERM] * kw[_PERM][:, None]).astype(np.float32)
    sink_h = (sin.T[_PERM] * (_SGN * kw[_PERM_SW])[:, None]).astype(np.float32)
    tab = np.zeros((128, 4, S), np.float32)
    tab[0:64, 0] = cosq_h
    tab[64:128, 0] = cosq_h
    tab[0:64, 1] = sinq_h
    tab[64:128, 1] = sinq_h
    tab[0:64, 2] = cosk_h
    tab[0:64, 3] = sink_h
    tab_b = tab.astype(ml_dtypes.bfloat16)

    masks_p = np.ascontiguousarray(
        mask_tiles.transpose(1, 0, 2)).astype(ml_dtypes.bfloat16)

    in_maps = []
    for c in range(8):
        b, g = c // 4, c % 4
        wq_s = Wq[:, g * 256:(g + 1) * 256].reshape(DIM, 4, 64)[:, :, _PERM]
        wq_s = wq_s.reshape(DIM, 256)
        wk_s = Wkv[:, g * HD:(g + 1) * HD][:, _PERM]
        wv_s = Wkv[:, KVH * HD + g * HD: KVH * HD + (g + 1) * HD]
        wkv_s = np.concatenate([wk_s, wv_s], axis=1)
        wo_s = Wo[g * 256:(g + 1) * 256, :]
        im = {
            "xp": np.ascontiguousarray(
                x[b].T.reshape(8, 128, N_CHUNKS, N_CHUNK)
                .transpose(1, 2, 0, 3)),
            "wq": np.ascontiguousarray(
                wq_s.reshape(8, 128, 256).transpose(1, 0, 2)),
            "wkv": np.ascontiguousarray(
                wkv_s.reshape(8, 128, 128).transpose(1, 0, 2)),
            "wo": np.ascontiguousarray(
                wo_s.reshape(2, 128, DIM).transpose(1, 0, 2)
            ).astype(ml_dtypes.bfloat16),
            "tab": tab_b,
            "masks": masks_p,
            "blockind": _BLOCKIND,
        }
        in_maps.append(im)

    from concourse.bass_utils import run_bass_kernel_spmd
    res = run_bass_kernel_spmd(nc, in_maps, core_ids=list(range(8)), trace=False)

    out = np.zeros((B, S, DIM), dtype=np.float32)
    for c in range(8):
        out[c // 4] += res.results[c]["outT"].T
    return out


# revision 23
# speedup vs baseline: 1.6150x; 1.3491x over previous
"""Block-causal GQA attention for Trainium2, 8 NeuronCores.

Sharding: core = (batch b, GQA group g): 2 batches x 4 kv-groups.
Each core computes its 4 q-heads + 1 kv-head on one batch element in a
"transposed" layout (head_dim on partitions, tokens on free dim), then a
row-parallel partial out-projection; the host sums the 4 partials per batch.

v2 layout/engine notes:
- Whole x resident in SBUF (64KB/partition), host-packed so every initial
  DMA is 128 contiguous per-partition descriptors.
- Scores run in bf16 (q~/k~ tiles bf16): 1 cy/row at any free size, keeps
  the PE HAM p-state fed; projections stay float32r.
- Head-dim stored pair-interleaved (perm[2j]=j, perm[2j+1]=j+32) so
  rotate_half becomes an adjacent-partition swap: one DVE stream_shuffle.
- All rsqrt/recip for RMS norms via scalar Ln->Exp (the natural_log_exp
  activation table also serves Exp/Square/Copy: zero table reloads).
- V transposed into (token, d) layout by the DMA xbar transpose engine.
- rope adds + mask multiplies on GpSimd (Pool); PSUM evacuation split
  DVE/scalar; denominator comes free as a 65th ones-row on V.
- Out-projection of chunk ci-1 is interleaved into the attention loop of
  chunk ci so PE/scalar never idle at phase boundaries.
"""
import numpy as np
import ml_dtypes

B, S, DIM = 2, 2048, 1024
H, KVH, HD = 16, 4, 64
EPS = 1e-6
SCALE = HD ** -0.5
PT_TILES = S // 128  # 16
N_CHUNK = 512
N_CHUNKS = S // N_CHUNK  # 4

_BUILD_CACHE = {}
_DEBUG = False
_BLOCKIND = np.zeros((2, 128), np.float32)
_BLOCKIND[0, 0:64] = 1.0
_BLOCKIND[1, 64:128] = 1.0

# pair-interleaved head-dim permutation: position 2j <- d j, 2j+1 <- d j+32
_PERM = np.empty(64, np.int64)
_PERM[0::2] = np.arange(32)
_PERM[1::2] = np.arange(32, 64)
_PERM_SW = _PERM[np.arange(64) ^ 1]          # partner (orig idx) per position
_SGN = np.where(np.arange(64) % 2 == 0, -1.0, 1.0).astype(np.float32)
_SHUF_MASK = [i ^ 1 for i in range(32)]


def _analyze_mask(mask):
    """Classify 128x128 tiles: 0=skip, 1=full, 2=mixed. Returns status grid,
    mixed tile stack (transposed to (k,q) layout, 0/1 float32), and index map.
    Index 0 of the stack is always the all-zero tile."""
    T = PT_TILES
    status = np.zeros((T, T), np.int8)
    tiles = [np.zeros((128, 128), np.float32)]
    idx = {}
    m = np.asarray(mask)
    for i in range(T):
        for j in range(T):
            sub = m[i * 128:(i + 1) * 128, j * 128:(j + 1) * 128]
            if not sub.any():
                status[i, j] = 0
            elif sub.all():
                status[i, j] = 1
            else:
                status[i, j] = 2
                idx[(i, j)] = len(tiles)
                tiles.append(np.ascontiguousarray(sub.T).astype(np.float32))
    return status, np.stack(tiles), idx


def _make_schedule(status, idx):
    """Per chunk: list of (ktile j, s0, s1, [(subtile s, mask_tile_index)])."""
    sched = []
    for ci in range(N_CHUNKS):
        qts = list(range(4 * ci, 4 * ci + 4))
        entries = []
        for j in range(PT_TILES):
            st = [status[i, j] for i in qts]
            if not any(st):
                continue
            alive = [s for s in range(4) if st[s] != 0]
            s0, s1 = alive[0], alive[-1] + 1
            mults = []
            for s in range(s0, s1):
                if st[s] == 1:
                    continue
                mults.append((s, 0 if st[s] == 0 else idx[(qts[s], j)]))
            entries.append((j, s0, s1, mults))
        sched.append(entries)
    return sched


def _build(sched_key, sched, n_masks, neg_c):
    import concourse.bacc as bacc
    import concourse.mybir as mybir
    import concourse.tile as tile

    F32 = mybir.dt.float32
    F32R = mybir.dt.float32r
    BF16 = mybir.dt.bfloat16
    LN = mybir.ActivationFunctionType.Ln
    EXPF = mybir.ActivationFunctionType.Exp

    nc = bacc.Bacc("TRN2", target_bir_lowering=False, debug=False)
    # host-packed dram tensors: every load is contiguous per partition
    xp = nc.dram_tensor("xp", (128, N_CHUNKS, 8, N_CHUNK), F32R,
                        kind="ExternalInput").ap()
    wq = nc.dram_tensor("wq", (128, 8, 256), F32R, kind="ExternalInput").ap()
    wkv = nc.dram_tensor("wkv", (128, 8, 128), F32R, kind="ExternalInput").ap()
    wo = nc.dram_tensor("wo", (128, 2, DIM), BF16, kind="ExternalInput").ap()
    tab = nc.dram_tensor("tab", (128, 4, S), BF16, kind="ExternalInput").ap()
    masks = nc.dram_tensor("masks", (128, n_masks, 128), BF16,
                           kind="ExternalInput").ap()
    blockind_d = nc.dram_tensor("blockind", (2, 128), F32R,
                                kind="ExternalInput").ap()
    outT = nc.dram_tensor("outT", (DIM, S), F32, kind="ExternalOutput").ap()
    if _DEBUG:
        dbg_t1a0 = nc.dram_tensor("dbg_t1a0", (128, S), BF16,
                                  kind="ExternalOutput").ap()
        dbg_kt2 = nc.dram_tensor("dbg_kt2", (128, S), BF16,
                                 kind="ExternalOutput").ap()
        dbg_vaug = nc.dram_tensor("dbg_vaug", (128, PT_TILES * 80), BF16,
                                  kind="ExternalOutput").ap()
        dbg_rkT = nc.dram_tensor("dbg_rkT", (128, 2 * PT_TILES), F32,
                                 kind="ExternalOutput").ap()
        dbg_attn = nc.dram_tensor("dbg_attn", (2, 128, N_CHUNK), BF16,
                                  kind="ExternalOutput").ap()
        dbg_pv = nc.dram_tensor("dbg_pv", (65, 2 * N_CHUNK), F32,
                                kind="ExternalOutput").ap()

    with tile.TileContext(nc) as tc:
        with tc.tile_pool(name="persist", bufs=1) as pp:
            # --- persistent tiles; DMA order = need order ----------------
            wq_sb = pp.tile([128, 8, 256], F32R)
            nc.sync.dma_start(out=wq_sb, in_=wq)
            xp_sb = pp.tile([128, N_CHUNKS, 8, N_CHUNK], F32R)
            nc.sync.dma_start(out=xp_sb[:, 0, 0:4], in_=xp[:, 0, 0:4])
            nc.sync.dma_start(out=xp_sb[:, 0, 4:8], in_=xp[:, 0, 4:8])
            tab_sb = pp.tile([128, 4, S], BF16)
            for ti in range(2):
                nc.sync.dma_start(out=tab_sb[:, ti], in_=tab[:, ti])
            nc.sync.dma_start(out=tab_sb[0:64, 2:4], in_=tab[0:64, 2:4])
            wkv_sb = pp.tile([128, 8, 128], F32R)
            nc.sync.dma_start(out=wkv_sb, in_=wkv)
            wo_sb = pp.tile([128, 2, DIM], BF16)
            nc.sync.dma_start(out=wo_sb, in_=wo)
            masks_sb = pp.tile([128, n_masks, 128], BF16)
            nc.sync.dma_start(out=masks_sb, in_=masks)
            blockind = pp.tile([2, 128], F32R)
            nc.sync.dma_start(out=blockind[:], in_=blockind_d)
            for ci in range(1, N_CHUNKS):
                nc.sync.dma_start(out=xp_sb[:, ci, 0:4], in_=xp[:, ci, 0:4])
                nc.sync.dma_start(out=xp_sb[:, ci, 4:8], in_=xp[:, ci, 4:8])

            t1a = [pp.tile([128, S], BF16, tag=f"t1a{m}", name=f"t1a{m}")
                   for m in range(2)]
            kt2 = pp.tile([128, S], BF16)
            v_aug = pp.tile([128, PT_TILES, 80], BF16)
            rkT = pp.tile([128, 2 * PT_TILES], F32)
            nrq = [pp.tile([2, S], F32, tag=f"nrq{m}", name=f"nrq{m}")
                   for m in range(2)]

            nc.vector.memset(v_aug[:, :, 64:65], 1.0)
            oq_f = pp.tile([128, 2], BF16)
            nc.vector.memset(oq_f, 0.0)
            nc.vector.memset(oq_f[0:64, 0:1], 1.0)
            nc.vector.memset(oq_f[64:128, 1:2], 1.0)
            ok_f = pp.tile([64, 2], BF16)
            nc.vector.memset(ok_f, 1.0)
            eps2 = pp.tile([2, 1], F32)
            nc.vector.memset(eps2, EPS)
            eps128 = pp.tile([128, 1], F32)
            nc.vector.memset(eps128, EPS)
            bias_c = pp.tile([128, 1], F32)
            nc.vector.memset(bias_c, neg_c)
            # pin the natural_log_exp activation table (serves Exp/Ln/Square/
            # Copy) so the table-load pass never thrashes between tables
            nc.scalar.add_instruction(mybir.InstLoadActFuncSet(
                name=nc.get_next_instruction_name(), ins=[], outs=[],
                act_func_set_id=6))

            with tc.tile_pool(name="p1", bufs=2) as p1, \
                 tc.tile_pool(name="p2", bufs=3) as p2, \
                 tc.tile_pool(name="p2s", bufs=2) as p2s, \
                 tc.tile_pool(name="p3", bufs=3) as p3, \
                 tc.tile_pool(name="pst", bufs=2, space="PSUM") as pst, \
                 tc.tile_pool(name="ppv", bufs=1, space="PSUM") as ppv, \
                 tc.tile_pool(name="pbig", bufs=2, space="PSUM") as pbig:

                def phase1a(ci):
                    off = ci * N_CHUNK
                    xt = xp_sb[:, ci]
                    nrm2 = pst.tile([2, 2, N_CHUNK], F32, tag="st",
                                    name=f"nrm2_{ci}")
                    for m in range(2):
                        q_ps = pbig.tile([128, N_CHUNK], F32, tag="big",
                                         name=f"qps{ci}_{m}")
                        for k in range(8):
                            nc.tensor.matmul(q_ps[:],
                                             wq_sb[:, k, m * 128:(m + 1) * 128],
                                             xt[:, k, :],
                                             start=(k == 0), stop=(k == 7))
                        qtr = p1.tile([128, N_CHUNK], BF16, tag="qtr",
                                      name=f"qtr{ci}_{m}")
                        nc.vector.tensor_copy(qtr[:], q_ps[:])
                        sqq = p1.tile([128, N_CHUNK], BF16, tag="sqq",
                                      name=f"sqq{ci}_{m}")
                        nc.scalar.square(sqq[:], q_ps[:])
                        nc.tensor.matmul(nrm2[:, m, :], oq_f[:], sqq[:],
                                         start=True, stop=True)
                        qrot = p1.tile([128, N_CHUNK], BF16, tag="qrot",
                                       name=f"qrot{ci}_{m}")
                        nc.vector.stream_shuffle(qrot[:], qtr[:], _SHUF_MASK)
                        tq = p1.tile([128, N_CHUNK], BF16, tag="tq",
                                     name=f"tq{ci}_{m}")
                        nc.vector.tensor_mul(tq[:], qtr[:],
                                             tab_sb[:, 0, off:off + N_CHUNK])
                        nc.vector.tensor_mul(qrot[:], qrot[:],
                                             tab_sb[:, 1, off:off + N_CHUNK])
                        nc.vector.tensor_add(t1a[m][:, off:off + N_CHUNK],
                                             tq[:], qrot[:])
                    nln = p1.tile([2, 2, N_CHUNK], F32, tag="nln",
                                  name=f"nln{ci}")
                    nc.scalar.activation(out=nln[:], in_=nrm2[:],
                                         func=LN, bias=eps2[:], scale=1.0 / HD)
                    nrq2 = p1.tile([2, 2, N_CHUNK], F32R, tag="nrq2",
                                   name=f"nrq2_{ci}")
                    nc.scalar.activation(out=nrq2[:], in_=nln[:],
                                         func=EXPF, scale=-0.5)

                    kv_ps = pbig.tile([128, N_CHUNK], F32, tag="big",
                                      name=f"kvps{ci}")
                    for k in range(8):
                        nc.tensor.matmul(kv_ps[:], wkv_sb[:, k, :], xt[:, k, :],
                                         start=(k == 0), stop=(k == 7))
                    ktr = p1.tile([64, N_CHUNK], BF16, tag="ktr",
                                  name=f"ktr{ci}")
                    nc.vector.tensor_copy(ktr[:], kv_ps[0:64, :])
                    vtr = p1.tile([64, N_CHUNK], BF16, tag="vtr",
                                  name=f"vtr{ci}")
                    nc.vector.tensor_copy(vtr[:], kv_ps[64:128, :])
                    sqk = p1.tile([64, N_CHUNK], BF16, tag="sqk",
                                  name=f"sqk{ci}")
                    nc.scalar.square(sqk[:], kv_ps[0:64, :])
                    nkT_ps = pbig.tile([128, 8], F32, tag="big",
                                       name=f"nkT{ci}")
                    for t in range(4):
                        nc.tensor.matmul(nkT_ps[:, 2 * t:2 * t + 2],
                                         sqk[:, t * 128:(t + 1) * 128],
                                         ok_f[:],
                                         start=(t == 0), stop=(t == 3))
                    rkS = p1.tile([128, 8], F32, tag="rkS", name=f"rkS{ci}")
                    nc.scalar.activation(out=rkS[:], in_=nkT_ps[:],
                                         func=LN, bias=eps128[:], scale=1.0 / HD)
                    nc.scalar.activation(out=rkT[:, 8 * ci:8 * ci + 8],
                                         in_=rkS[:], func=EXPF, scale=-0.5)
                    krot = p1.tile([64, N_CHUNK], BF16, tag="krot",
                                   name=f"krot{ci}")
                    nc.vector.stream_shuffle(krot[:], ktr[:], _SHUF_MASK)
                    k1 = p1.tile([64, N_CHUNK], BF16, tag="k1", name=f"k1{ci}")
                    nc.vector.tensor_mul(k1[:], ktr[:],
                                         tab_sb[0:64, 2, off:off + N_CHUNK])
                    nc.vector.tensor_mul(krot[:], krot[:],
                                         tab_sb[0:64, 3, off:off + N_CHUNK])
                    nc.vector.tensor_add(kt2[0:64, off:off + N_CHUNK],
                                         k1[:], krot[:])
                    nc.sync.dma_start(out=kt2[64:128, off:off + N_CHUNK],
                                      in_=kt2[0:64, off:off + N_CHUNK])
                    for t in range(4):
                        nc.sync.dma_start_transpose(
                            v_aug[:, 4 * ci + t, 0:64],
                            vtr[:, t * 128:(t + 1) * 128])
                    return nrq2

                def phase1b(ci, nrq2):
                    off = ci * N_CHUNK
                    for m in range(2):
                        rep_ps = pbig.tile([128, N_CHUNK], F32, tag="big",
                                           name=f"repps{ci}_{m}")
                        nc.tensor.matmul(rep_ps[:], blockind[:],
                                         nrq2[:, m, :],
                                         start=True, stop=True)
                        nc.vector.tensor_mul(
                            t1a[m][:, off:off + N_CHUNK],
                            t1a[m][:, off:off + N_CHUNK], rep_ps[:])

                def phase3_mo(ci, attn_ts, mo):
                    off = ci * N_CHUNK
                    o_ps = pbig.tile([128, N_CHUNK], F32, tag="big",
                                     name=f"ops{ci}_{mo}")
                    for k2_ in range(2):
                        nc.tensor.matmul(o_ps[:],
                                         wo_sb[:, k2_, mo * 128:(mo + 1) * 128],
                                         attn_ts[k2_][:],
                                         start=(k2_ == 0), stop=(k2_ == 1))
                    o_sb = p3.tile([128, N_CHUNK], F32, tag="osb",
                                   name=f"osb{ci}_{mo}")
                    nc.vector.tensor_copy(o_sb[:], o_ps[:])
                    nc.scalar.dma_start(
                        out=outT[mo * 128:(mo + 1) * 128, off:off + N_CHUNK],
                        in_=o_sb[:])

                def phase2(m, ci, deferred):
                    off = ci * N_CHUNK
                    entries = sched[ci]
                    attn_c = p2s.tile([128, N_CHUNK], BF16, tag=f"attn{m}",
                                      name=f"attn{m}_{ci}")
                    pv = ppv.tile([65, 2, N_CHUNK], F32, tag="pv",
                                  name=f"pv{m}_{ci}")
                    for idx_e, (j, s0, s1, mults) in enumerate(entries):
                        koff = j * 128
                        a, b_ = s0 * 128, s1 * 128
                        st = pst.tile([128, 2, N_CHUNK], F32, tag="st",
                                      name=f"st{m}_{ci}_{j}")
                        nc.tensor.matmul(
                            st[:, 0, a:b_],
                            kt2[0:64, koff:koff + 128],
                            t1a[m][0:64, off + a:off + b_],
                            start=True, stop=True)
                        nc.tensor.matmul(
                            st[:, 1, a:b_],
                            kt2[64:128, koff:koff + 128],
                            t1a[m][64:128, off + a:off + b_],
                            start=True, stop=True, tile_position=(64, 0))
                        pt = p2.tile([128, 2, N_CHUNK], BF16, tag="pt",
                                     name=f"pt{m}_{ci}_{j}")
                        nc.scalar.activation(
                            out=pt[:, :, a:b_], in_=st[:, :, a:b_],
                            func=EXPF,
                            bias=bias_c[:], scale=rkT[:, 2 * j:2 * j + 1])
                        for s_, mt in mults:
                            nc.vector.tensor_mul(
                                pt[:, :, s_ * 128:(s_ + 1) * 128],
                                pt[:, :, s_ * 128:(s_ + 1) * 128],
                                masks_sb[:, mt:mt + 1, :].broadcast_to(
                                    [128, 2, 128]))
                        first = (idx_e == 0)
                        last = (idx_e == len(entries) - 1)
                        for hh in range(2):
                            nc.tensor.matmul(pv[:, hh, a:b_],
                                             v_aug[:, j, 0:65],
                                             pt[:, hh, a:b_],
                                             start=first, stop=last)
                        # interleave previous chunk's out-projection so the
                        # PE/DVE keep busy while scalar exp paces this loop
                        if idx_e >= 1 and deferred:
                            phase3_mo(*deferred.pop(0))
                    dsb = p2s.tile([1, 2, N_CHUNK], F32, tag="dsb",
                                   name=f"dsb{m}_{ci}")
                    nc.vector.tensor_copy(dsb[:], pv[64:65, :, :])
                    rd = p2s.tile([1, 2, N_CHUNK], F32, tag="rd",
                                  name=f"rd{m}_{ci}")
                    nc.vector.reciprocal_approx_fast(out=rd[:], in_=dsb[:])
                    bcd = p2s.tile([64, 2, N_CHUNK], F32, tag="bcd", bufs=1,
                                   name=f"bcd{m}_{ci}")
                    nc.gpsimd.partition_broadcast(bcd[:], rd[:], channels=64)
                    for hh in range(2):
                        nc.vector.tensor_mul(
                            attn_c[hh * 64:(hh + 1) * 64, :],
                            pv[0:64, hh, :], bcd[:, hh, :])
                    if _DEBUG and m == 0 and ci == 0:
                        pvs = p2s.tile([65, 2, N_CHUNK], F32, tag="dbgpv",
                                       name="dbgpv")
                        nc.vector.tensor_copy(pvs[:], pv[:])
                        nc.sync.dma_start(
                            out=dbg_pv,
                            in_=pvs.rearrange("p a b -> p (a b)"))
                    return attn_c

                prev = None
                for ci in range(N_CHUNKS):
                    nrq2 = phase1a(ci)
                    phase1b(ci, nrq2)
                    deferred = []
                    if prev is not None:
                        pci, a0p, a1p = prev
                        deferred = [(pci, (a0p, a1p), mo) for mo in range(8)]
                    a0 = phase2(0, ci, deferred)
                    a1 = phase2(1, ci, deferred)
                    for d in deferred:
                        phase3_mo(*d)
                    prev = (ci, a0, a1)
                pci, a0p, a1p = prev
                for mo in range(8):
                    phase3_mo(pci, (a0p, a1p), mo)
                if _DEBUG:
                    nc.sync.dma_start(out=dbg_attn[0], in_=a0p[:])
                    nc.sync.dma_start(out=dbg_attn[1], in_=a1p[:])
                    nc.sync.dma_start(out=dbg_t1a0, in_=t1a[0][:])
                    nc.sync.dma_start(out=dbg_kt2, in_=kt2[:])
                    nc.sync.dma_start(
                        out=dbg_vaug,
                        in_=v_aug.rearrange("p a b -> p (a b)"))
                    nc.sync.dma_start(out=dbg_rkT, in_=rkT[:])

    nc.compile()
    return nc


def _get_nc(sched_key, sched, n_masks, neg_c):
    key = (sched_key, n_masks, float(neg_c))
    if key not in _BUILD_CACHE:
        _BUILD_CACHE[key] = _build(sched_key, sched, n_masks, neg_c)
    return _BUILD_CACHE[key]


def kernel(x, Wq, Wkv, Wo, q_norm_w, k_norm_w, rope_cos, rope_sin,
           attention_mask):
    x = np.asarray(x, dtype=np.float32)
    Wq = np.asarray(Wq, dtype=np.float32)
    Wkv = np.asarray(Wkv, dtype=np.float32)
    Wo = np.asarray(Wo, dtype=np.float32)
    qw = np.asarray(q_norm_w, dtype=np.float32)
    kw = np.asarray(k_norm_w, dtype=np.float32)
    cos = np.asarray(rope_cos, dtype=np.float32)
    sin = np.asarray(rope_sin, dtype=np.float32)

    status, mask_tiles, idx = _analyze_mask(attention_mask)
    sched = _make_schedule(status, idx)
    sched_key = status.tobytes()

    # numerically safe exp shift (0 in the normal regime)
    mct_q = max(np.abs(cos).max(), np.abs(sin).max(), 1e-9)
    bound = SCALE * 2.0 * HD * mct_q * mct_q \
        * max(np.abs(qw).max(), 1e-9) * max(np.abs(kw).max(), 1e-9)
    neg_c = -max(0.0, float(bound) - 60.0)

    nc = _get_nc(sched_key, sched, mask_tiles.shape[0], neg_c)

    # host-folded rope tables in the pair-interleaved d layout
    cosq_h = (cos.T[_PERM] * (qw[_PERM] * SCALE)[:, None]).astype(np.float32)
    sinq_h = (sin.T[_PERM] * (_SGN * qw[_PERM_SW] * SCALE)[:, None]
              ).astype(np.float32)
    cosk_h = (cos.T[_PERM] * kw[_PERM][:, None]).astype(np.float32)
    sink_h = (sin.T[_PERM] * (_SGN * kw[_PERM_SW])[:, None]).astype(np.float32)
    tab = np.zeros((128, 4, S), np.float32)
    tab[0:64, 0] = cosq_h
    tab[64:128, 0] = cosq_h
    tab[0:64, 1] = sinq_h
    tab[64:128, 1] = sinq_h
    tab[0:64, 2] = cosk_h
    tab[0:64, 3] = sink_h
    tab_b = tab.astype(ml_dtypes.bfloat16)

    masks_p = np.ascontiguousarray(
        mask_tiles.transpose(1, 0, 2)).astype(ml_dtypes.bfloat16)

    in_maps = []
    for c in range(8):
        b, g = c // 4, c % 4
        wq_s = Wq[:, g * 256:(g + 1) * 256].reshape(DIM, 4, 64)[:, :, _PERM]
        wq_s = wq_s.reshape(DIM, 256)
        wk_s = Wkv[:, g * HD:(g + 1) * HD][:, _PERM]
        wv_s = Wkv[:, KVH * HD + g * HD: KVH * HD + (g + 1) * HD]
        wkv_s = np.concatenate([wk_s, wv_s], axis=1)
        wo_s = Wo[g * 256:(g + 1) * 256, :]
        im = {
            "xp": np.ascontiguousarray(
                x[b].T.reshape(8, 128, N_CHUNKS, N_CHUNK)
                .transpose(1, 2, 0, 3)),
            "wq": np.ascontiguousarray(
                wq_s.reshape(8, 128, 256).transpose(1, 0, 2)),
            "wkv": np.ascontiguousarray(
                wkv_s.reshape(8, 128, 128).transpose(1, 0, 2)),
            "wo": np.ascontiguousarray(
                wo_s.reshape(2, 128, DIM).transpose(1, 0, 2)
            ).astype(ml_dtypes.bfloat16),
            "tab": tab_b,
            "masks": masks_p,
            "blockind": _BLOCKIND,
        }
        in_maps.append(im)

    from concourse.bass_utils import run_bass_kernel_spmd
    res = run_bass_kernel_spmd(nc, in_maps, core_ids=list(range(8)), trace=False)

    out = np.zeros((B, S, DIM), dtype=np.float32)
    for c in range(8):
        out[c // 4] += res.results[c]["outT"].T
    return out


# revision 24
# speedup vs baseline: 1.6244x; 1.0058x over previous
"""Block-causal GQA attention for Trainium2, 8 NeuronCores.

Sharding: core = (batch b, GQA group g): 2 batches x 4 kv-groups.
Each core computes its 4 q-heads + 1 kv-head on one batch element in a
"transposed" layout (head_dim on partitions, tokens on free dim), then a
row-parallel partial out-projection; the host sums the 4 partials per batch.

v2 layout/engine notes:
- Whole x resident in SBUF (64KB/partition), host-packed so every initial
  DMA is 128 contiguous per-partition descriptors.
- Scores run in bf16 (q~/k~ tiles bf16): 1 cy/row at any free size, keeps
  the PE HAM p-state fed; projections stay float32r.
- Head-dim stored pair-interleaved (perm[2j]=j, perm[2j+1]=j+32) so
  rotate_half becomes an adjacent-partition swap: one DVE stream_shuffle.
- All rsqrt/recip for RMS norms via scalar Ln->Exp (the natural_log_exp
  activation table also serves Exp/Square/Copy: zero table reloads).
- V transposed into (token, d) layout by the DMA xbar transpose engine.
- rope adds + mask multiplies on GpSimd (Pool); PSUM evacuation split
  DVE/scalar; denominator comes free as a 65th ones-row on V.
- Out-projection of chunk ci-1 is interleaved into the attention loop of
  chunk ci so PE/scalar never idle at phase boundaries.
"""
import numpy as np
import ml_dtypes

B, S, DIM = 2, 2048, 1024
H, KVH, HD = 16, 4, 64
EPS = 1e-6
SCALE = HD ** -0.5
PT_TILES = S // 128  # 16
N_CHUNK = 512
N_CHUNKS = S // N_CHUNK  # 4

_BUILD_CACHE = {}
_DEBUG = False
_BLOCKIND = np.zeros((2, 128), np.float32)
_BLOCKIND[0, 0:64] = 1.0
_BLOCKIND[1, 64:128] = 1.0

# pair-interleaved head-dim permutation: position 2j <- d j, 2j+1 <- d j+32
_PERM = np.empty(64, np.int64)
_PERM[0::2] = np.arange(32)
_PERM[1::2] = np.arange(32, 64)
_PERM_SW = _PERM[np.arange(64) ^ 1]          # partner (orig idx) per position
_SGN = np.where(np.arange(64) % 2 == 0, -1.0, 1.0).astype(np.float32)
_SHUF_MASK = [i ^ 1 for i in range(32)]


def _analyze_mask(mask):
    """Classify 128x128 tiles: 0=skip, 1=full, 2=mixed. Returns status grid,
    mixed tile stack (transposed to (k,q) layout, 0/1 float32), and index map.
    Index 0 of the stack is always the all-zero tile."""
    T = PT_TILES
    status = np.zeros((T, T), np.int8)
    tiles = [np.zeros((128, 128), np.float32)]
    idx = {}
    m = np.asarray(mask)
    for i in range(T):
        for j in range(T):
            sub = m[i * 128:(i + 1) * 128, j * 128:(j + 1) * 128]
            if not sub.any():
                status[i, j] = 0
            elif sub.all():
                status[i, j] = 1
            else:
                status[i, j] = 2
                idx[(i, j)] = len(tiles)
                tiles.append(np.ascontiguousarray(sub.T).astype(np.float32))
    return status, np.stack(tiles), idx


def _make_schedule(status, idx):
    """Per chunk: list of (ktile j, s0, s1, [(subtile s, mask_tile_index)])."""
    sched = []
    for ci in range(N_CHUNKS):
        qts = list(range(4 * ci, 4 * ci + 4))
        entries = []
        for j in range(PT_TILES):
            st = [status[i, j] for i in qts]
            if not any(st):
                continue
            alive = [s for s in range(4) if st[s] != 0]
            s0, s1 = alive[0], alive[-1] + 1
            mults = []
            for s in range(s0, s1):
                if st[s] == 1:
                    continue
                mults.append((s, 0 if st[s] == 0 else idx[(qts[s], j)]))
            entries.append((j, s0, s1, mults))
        sched.append(entries)
    return sched


def _build(sched_key, sched, n_masks, neg_c):
    import concourse.bacc as bacc
    import concourse.mybir as mybir
    import concourse.tile as tile

    F32 = mybir.dt.float32
    F32R = mybir.dt.float32r
    BF16 = mybir.dt.bfloat16
    LN = mybir.ActivationFunctionType.Ln
    EXPF = mybir.ActivationFunctionType.Exp

    nc = bacc.Bacc("TRN2", target_bir_lowering=False, debug=False)
    # host-packed dram tensors: every load is contiguous per partition
    xp = nc.dram_tensor("xp", (128, N_CHUNKS, 8, N_CHUNK), F32R,
                        kind="ExternalInput").ap()
    wq = nc.dram_tensor("wq", (128, 8, 256), F32R, kind="ExternalInput").ap()
    wkv = nc.dram_tensor("wkv", (128, 8, 128), F32R, kind="ExternalInput").ap()
    wo = nc.dram_tensor("wo", (128, 2, DIM), BF16, kind="ExternalInput").ap()
    tab = nc.dram_tensor("tab", (128, 4, S), BF16, kind="ExternalInput").ap()
    masks = nc.dram_tensor("masks", (128, n_masks, 128), BF16,
                           kind="ExternalInput").ap()
    blockind_d = nc.dram_tensor("blockind", (2, 128), F32R,
                                kind="ExternalInput").ap()
    outT = nc.dram_tensor("outT", (DIM, S), F32, kind="ExternalOutput").ap()
    if _DEBUG:
        dbg_t1a0 = nc.dram_tensor("dbg_t1a0", (128, S), BF16,
                                  kind="ExternalOutput").ap()
        dbg_kt2 = nc.dram_tensor("dbg_kt2", (128, S), BF16,
                                 kind="ExternalOutput").ap()
        dbg_vaug = nc.dram_tensor("dbg_vaug", (128, PT_TILES * 80), BF16,
                                  kind="ExternalOutput").ap()
        dbg_rkT = nc.dram_tensor("dbg_rkT", (128, 2 * PT_TILES), F32,
                                 kind="ExternalOutput").ap()
        dbg_attn = nc.dram_tensor("dbg_attn", (2, 128, N_CHUNK), BF16,
                                  kind="ExternalOutput").ap()
        dbg_pv = nc.dram_tensor("dbg_pv", (65, 2 * N_CHUNK), F32,
                                kind="ExternalOutput").ap()

    with tile.TileContext(nc) as tc:
        with tc.tile_pool(name="persist", bufs=1) as pp:
            # --- persistent tiles; DMA order = need order ----------------
            wq_sb = pp.tile([128, 8, 256], F32R)
            xp_sb = pp.tile([128, N_CHUNKS, 8, N_CHUNK], F32R)
            for k in range(8):
                nc.sync.dma_start(out=wq_sb[:, k], in_=wq[:, k])
                nc.sync.dma_start(out=xp_sb[:, 0, k], in_=xp[:, 0, k])
            tab_sb = pp.tile([128, 4, S], BF16)
            for ti in range(2):
                nc.sync.dma_start(out=tab_sb[:, ti], in_=tab[:, ti])
            nc.sync.dma_start(out=tab_sb[0:64, 2:4], in_=tab[0:64, 2:4])
            wkv_sb = pp.tile([128, 8, 128], F32R)
            nc.sync.dma_start(out=wkv_sb, in_=wkv)
            wo_sb = pp.tile([128, 2, DIM], BF16)
            nc.sync.dma_start(out=wo_sb, in_=wo)
            masks_sb = pp.tile([128, n_masks, 128], BF16)
            nc.sync.dma_start(out=masks_sb, in_=masks)
            blockind = pp.tile([2, 128], F32R)
            nc.sync.dma_start(out=blockind[:], in_=blockind_d)
            for ci in range(1, N_CHUNKS):
                nc.sync.dma_start(out=xp_sb[:, ci, 0:4], in_=xp[:, ci, 0:4])
                nc.sync.dma_start(out=xp_sb[:, ci, 4:8], in_=xp[:, ci, 4:8])

            t1a = [pp.tile([128, S], BF16, tag=f"t1a{m}", name=f"t1a{m}")
                   for m in range(2)]
            kt2 = pp.tile([128, S], BF16)
            v_aug = pp.tile([128, PT_TILES, 80], BF16)
            rkT = pp.tile([128, 2 * PT_TILES], F32)
            nrq = [pp.tile([2, S], F32, tag=f"nrq{m}", name=f"nrq{m}")
                   for m in range(2)]

            nc.vector.memset(v_aug[:, :, 64:65], 1.0)
            oq_f = pp.tile([128, 2], BF16)
            nc.vector.memset(oq_f, 0.0)
            nc.vector.memset(oq_f[0:64, 0:1], 1.0)
            nc.vector.memset(oq_f[64:128, 1:2], 1.0)
            ok_f = pp.tile([64, 2], BF16)
            nc.vector.memset(ok_f, 1.0)
            eps2 = pp.tile([2, 1], F32)
            nc.vector.memset(eps2, EPS)
            eps128 = pp.tile([128, 1], F32)
            nc.vector.memset(eps128, EPS)
            bias_c = pp.tile([128, 1], F32)
            nc.vector.memset(bias_c, neg_c)
            # pin the natural_log_exp activation table (serves Exp/Ln/Square/
            # Copy) so the table-load pass never thrashes between tables
            nc.scalar.add_instruction(mybir.InstLoadActFuncSet(
                name=nc.get_next_instruction_name(), ins=[], outs=[],
                act_func_set_id=6))

            with tc.tile_pool(name="p1", bufs=2) as p1, \
                 tc.tile_pool(name="p2", bufs=3) as p2, \
                 tc.tile_pool(name="p2s", bufs=2) as p2s, \
                 tc.tile_pool(name="p3", bufs=3) as p3, \
                 tc.tile_pool(name="pst", bufs=2, space="PSUM") as pst, \
                 tc.tile_pool(name="ppv", bufs=1, space="PSUM") as ppv, \
                 tc.tile_pool(name="pbig", bufs=2, space="PSUM") as pbig:

                def phase1a(ci):
                    off = ci * N_CHUNK
                    xt = xp_sb[:, ci]
                    nrm2 = pst.tile([2, 2, N_CHUNK], F32, tag="st",
                                    name=f"nrm2_{ci}")
                    for m in range(2):
                        q_ps = pbig.tile([128, N_CHUNK], F32, tag="big",
                                         name=f"qps{ci}_{m}")
                        for k in range(8):
                            nc.tensor.matmul(q_ps[:],
                                             wq_sb[:, k, m * 128:(m + 1) * 128],
                                             xt[:, k, :],
                                             start=(k == 0), stop=(k == 7))
                        qtr = p1.tile([128, N_CHUNK], BF16, tag="qtr",
                                      name=f"qtr{ci}_{m}")
                        nc.vector.tensor_copy(qtr[:], q_ps[:])
                        sqq = p1.tile([128, N_CHUNK], BF16, tag="sqq",
                                      name=f"sqq{ci}_{m}")
                        nc.scalar.square(sqq[:], q_ps[:])
                        nc.tensor.matmul(nrm2[:, m, :], oq_f[:], sqq[:],
                                         start=True, stop=True)
                        qrot = p1.tile([128, N_CHUNK], BF16, tag="qrot",
                                       name=f"qrot{ci}_{m}")
                        nc.vector.stream_shuffle(qrot[:], qtr[:], _SHUF_MASK)
                        tq = p1.tile([128, N_CHUNK], BF16, tag="tq",
                                     name=f"tq{ci}_{m}")
                        nc.vector.tensor_mul(tq[:], qtr[:],
                                             tab_sb[:, 0, off:off + N_CHUNK])
                        nc.vector.tensor_mul(qrot[:], qrot[:],
                                             tab_sb[:, 1, off:off + N_CHUNK])
                        nc.vector.tensor_add(t1a[m][:, off:off + N_CHUNK],
                                             tq[:], qrot[:])
                    nln = p1.tile([2, 2, N_CHUNK], F32, tag="nln",
                                  name=f"nln{ci}")
                    nc.scalar.activation(out=nln[:], in_=nrm2[:],
                                         func=LN, bias=eps2[:], scale=1.0 / HD)
                    nrq2 = p1.tile([2, 2, N_CHUNK], F32R, tag="nrq2",
                                   name=f"nrq2_{ci}")
                    nc.scalar.activation(out=nrq2[:], in_=nln[:],
                                         func=EXPF, scale=-0.5)

                    kv_ps = pbig.tile([128, N_CHUNK], F32, tag="big",
                                      name=f"kvps{ci}")
                    for k in range(8):
                        nc.tensor.matmul(kv_ps[:], wkv_sb[:, k, :], xt[:, k, :],
                                         start=(k == 0), stop=(k == 7))
                    ktr = p1.tile([64, N_CHUNK], BF16, tag="ktr",
                                  name=f"ktr{ci}")
                    nc.vector.tensor_copy(ktr[:], kv_ps[0:64, :])
                    vtr = p1.tile([64, N_CHUNK], BF16, tag="vtr",
                                  name=f"vtr{ci}")
                    nc.vector.tensor_copy(vtr[:], kv_ps[64:128, :])
                    sqk = p1.tile([64, N_CHUNK], BF16, tag="sqk",
                                  name=f"sqk{ci}")
                    nc.scalar.square(sqk[:], kv_ps[0:64, :])
                    nkT_ps = pbig.tile([128, 8], F32, tag="big",
                                       name=f"nkT{ci}")
                    for t in range(4):
                        nc.tensor.matmul(nkT_ps[:, 2 * t:2 * t + 2],
                                         sqk[:, t * 128:(t + 1) * 128],
                                         ok_f[:],
                                         start=(t == 0), stop=(t == 3))
                    rkS = p1.tile([128, 8], F32, tag="rkS", name=f"rkS{ci}")
                    nc.scalar.activation(out=rkS[:], in_=nkT_ps[:],
                                         func=LN, bias=eps128[:], scale=1.0 / HD)
                    nc.scalar.activation(out=rkT[:, 8 * ci:8 * ci + 8],
                                         in_=rkS[:], func=EXPF, scale=-0.5)
                    krot = p1.tile([64, N_CHUNK], BF16, tag="krot",
                                   name=f"krot{ci}")
                    nc.vector.stream_shuffle(krot[:], ktr[:], _SHUF_MASK)
                    k1 = p1.tile([64, N_CHUNK], BF16, tag="k1", name=f"k1{ci}")
                    nc.vector.tensor_mul(k1[:], ktr[:],
                                         tab_sb[0:64, 2, off:off + N_CHUNK])
                    nc.vector.tensor_mul(krot[:], krot[:],
                                         tab_sb[0:64, 3, off:off + N_CHUNK])
                    nc.vector.tensor_add(kt2[0:64, off:off + N_CHUNK],
                                         k1[:], krot[:])
                    nc.sync.dma_start(out=kt2[64:128, off:off + N_CHUNK],
                                      in_=kt2[0:64, off:off + N_CHUNK])
                    for t in range(4):
                        nc.sync.dma_start_transpose(
                            v_aug[:, 4 * ci + t, 0:64],
                            vtr[:, t * 128:(t + 1) * 128])
                    return nrq2

                def phase1b(ci, nrq2):
                    off = ci * N_CHUNK
                    for m in range(2):
                        rep_ps = pbig.tile([128, N_CHUNK], F32, tag="big",
                                           name=f"repps{ci}_{m}")
                        nc.tensor.matmul(rep_ps[:], blockind[:],
                                         nrq2[:, m, :],
                                         start=True, stop=True)
                        nc.vector.tensor_mul(
                            t1a[m][:, off:off + N_CHUNK],
                            t1a[m][:, off:off + N_CHUNK], rep_ps[:])

                def phase3_mo(ci, attn_ts, mo):
                    off = ci * N_CHUNK
                    o_ps = pbig.tile([128, N_CHUNK], F32, tag="big",
                                     name=f"ops{ci}_{mo}")
                    for k2_ in range(2):
                        nc.tensor.matmul(o_ps[:],
                                         wo_sb[:, k2_, mo * 128:(mo + 1) * 128],
                                         attn_ts[k2_][:],
                                         start=(k2_ == 0), stop=(k2_ == 1))
                    o_sb = p3.tile([128, N_CHUNK], F32, tag="osb",
                                   name=f"osb{ci}_{mo}")
                    nc.vector.tensor_copy(o_sb[:], o_ps[:])
                    nc.scalar.dma_start(
                        out=outT[mo * 128:(mo + 1) * 128, off:off + N_CHUNK],
                        in_=o_sb[:])

                def phase2(m, ci, deferred):
                    off = ci * N_CHUNK
                    entries = sched[ci]
                    attn_c = p2s.tile([128, N_CHUNK], BF16, tag=f"attn{m}",
                                      name=f"attn{m}_{ci}")
                    pv = ppv.tile([65, 2, N_CHUNK], F32, tag="pv",
                                  name=f"pv{m}_{ci}")
                    for idx_e, (j, s0, s1, mults) in enumerate(entries):
                        koff = j * 128
                        a, b_ = s0 * 128, s1 * 128
                        st = pst.tile([128, 2, N_CHUNK], F32, tag="st",
                                      name=f"st{m}_{ci}_{j}")
                        nc.tensor.matmul(
                            st[:, 0, a:b_],
                            kt2[0:64, koff:koff + 128],
                            t1a[m][0:64, off + a:off + b_],
                            start=True, stop=True)
                        nc.tensor.matmul(
                            st[:, 1, a:b_],
                            kt2[64:128, koff:koff + 128],
                            t1a[m][64:128, off + a:off + b_],
                            start=True, stop=True, tile_position=(64, 0))
                        pt = p2.tile([128, 2, N_CHUNK], BF16, tag="pt",
                                     name=f"pt{m}_{ci}_{j}")
                        nc.scalar.activation(
                            out=pt[:, :, a:b_], in_=st[:, :, a:b_],
                            func=EXPF,
                            bias=bias_c[:], scale=rkT[:, 2 * j:2 * j + 1])
                        for s_, mt in mults:
                            nc.vector.tensor_mul(
                                pt[:, :, s_ * 128:(s_ + 1) * 128],
                                pt[:, :, s_ * 128:(s_ + 1) * 128],
                                masks_sb[:, mt:mt + 1, :].broadcast_to(
                                    [128, 2, 128]))
                        first = (idx_e == 0)
                        last = (idx_e == len(entries) - 1)
                        for hh in range(2):
                            nc.tensor.matmul(pv[:, hh, a:b_],
                                             v_aug[:, j, 0:65],
                                             pt[:, hh, a:b_],
                                             start=first, stop=last)
                        # interleave previous chunk's out-projection so the
                        # PE/DVE keep busy while scalar exp paces this loop
                        if idx_e >= 1 and deferred:
                            phase3_mo(*deferred.pop(0))
                    dsb = p2s.tile([1, 2, N_CHUNK], F32, tag="dsb",
                                   name=f"dsb{m}_{ci}")
                    nc.scalar.copy(dsb[:], pv[64:65, :, :])
                    rd = p2s.tile([1, 2, N_CHUNK], F32, tag="rd",
                                  name=f"rd{m}_{ci}")
                    nc.vector.reciprocal_approx_fast(out=rd[:], in_=dsb[:])
                    bcd = p2s.tile([64, 2, N_CHUNK], F32, tag="bcd", bufs=1,
                                   name=f"bcd{m}_{ci}")
                    nc.gpsimd.partition_broadcast(bcd[:], rd[:], channels=64)
                    for hh in range(2):
                        nc.vector.tensor_mul(
                            attn_c[hh * 64:(hh + 1) * 64, :],
                            pv[0:64, hh, :], bcd[:, hh, :])
                    if _DEBUG and m == 0 and ci == 0:
                        pvs = p2s.tile([65, 2, N_CHUNK], F32, tag="dbgpv",
                                       name="dbgpv")
                        nc.vector.tensor_copy(pvs[:], pv[:])
                        nc.sync.dma_start(
                            out=dbg_pv,
                            in_=pvs.rearrange("p a b -> p (a b)"))
                    return attn_c

                prev = None
                for ci in range(N_CHUNKS):
                    nrq2 = phase1a(ci)
                    phase1b(ci, nrq2)
                    deferred = []
                    if prev is not None:
                        pci, a0p, a1p = prev
                        deferred = [(pci, (a0p, a1p), mo) for mo in range(8)]
                    a0 = phase2(0, ci, deferred)
                    a1 = phase2(1, ci, deferred)
                    for d in deferred:
                        phase3_mo(*d)
                    prev = (ci, a0, a1)
                pci, a0p, a1p = prev
                for mo in range(8):
                    phase3_mo(pci, (a0p, a1p), mo)
                if _DEBUG:
                    nc.sync.dma_start(out=dbg_attn[0], in_=a0p[:])
                    nc.sync.dma_start(out=dbg_attn[1], in_=a1p[:])
                    nc.sync.dma_start(out=dbg_t1a0, in_=t1a[0][:])
                    nc.sync.dma_start(out=dbg_kt2, in_=kt2[:])
                    nc.sync.dma_start(
                        out=dbg_vaug,
                        in_=v_aug.rearrange("p a b -> p (a b)"))
                    nc.sync.dma_start(out=dbg_rkT, in_=rkT[:])

    nc.compile()
    return nc


def _get_nc(sched_key, sched, n_masks, neg_c):
    key = (sched_key, n_masks, float(neg_c))
    if key not in _BUILD_CACHE:
        _BUILD_CACHE[key] = _build(sched_key, sched, n_masks, neg_c)
    return _BUILD_CACHE[key]


def kernel(x, Wq, Wkv, Wo, q_norm_w, k_norm_w, rope_cos, rope_sin,
           attention_mask):
    x = np.asarray(x, dtype=np.float32)
    Wq = np.asarray(Wq, dtype=np.float32)
    Wkv = np.asarray(Wkv, dtype=np.float32)
    Wo = np.asarray(Wo, dtype=np.float32)
    qw = np.asarray(q_norm_w, dtype=np.float32)
    kw = np.asarray(k_norm_w, dtype=np.float32)
    cos = np.asarray(rope_cos, dtype=np.float32)
    sin = np.asarray(rope_sin, dtype=np.float32)

    status, mask_tiles, idx = _analyze_mask(attention_mask)
    sched = _make_schedule(status, idx)
    sched_key = status.tobytes()

    # numerically safe exp shift (0 in the normal regime)
    mct_q = max(np.abs(cos).max(), np.abs(sin).max(), 1e-9)
    bound = SCALE * 2.0 * HD * mct_q * mct_q \
        * max(np.abs(qw).max(), 1e-9) * max(np.abs(kw).max(), 1e-9)
    neg_c = -max(0.0, float(bound) - 60.0)

    nc = _get_nc(sched_key, sched, mask_tiles.shape[0], neg_c)

    # host-folded rope tables in the pair-interleaved d layout
    cosq_h = (cos.T[_PERM] * (qw[_PERM] * SCALE)[:, None]).astype(np.float32)
    sinq_h = (sin.T[_PERM] * (_SGN * qw[_PERM_SW] * SCALE)[:, None]
              ).astype(np.float32)
    cosk_h = (cos.T[_PERM] * kw[_PERM][:, None]).astype(np.float32)
    sink_h = (sin.T[_PERM] * (_SGN * kw[_PERM_SW])[:, None]).astype(np.float32)
    tab = np.zeros((128, 4, S), np.float32)
    tab[0:64, 0] = cosq_h
    tab[64:128, 0] = cosq_h
    tab[0:64, 1] = sinq_h
    tab[64:128, 1] = sinq_h
    tab[0:64, 2] = cosk_h
    tab[0:64, 3] = sink_h
    tab_b = tab.astype(ml_dtypes.bfloat16)

    masks_p = np.ascontiguousarray(
        mask_tiles.transpose(1, 0, 2)).astype(ml_dtypes.bfloat16)

    in_maps = []
    for c in range(8):
        b, g = c // 4, c % 4
        wq_s = Wq[:, g * 256:(g + 1) * 256].reshape(DIM, 4, 64)[:, :, _PERM]
        wq_s = wq_s.reshape(DIM, 256)
        wk_s = Wkv[:, g * HD:(g + 1) * HD][:, _PERM]
        wv_s = Wkv[:, KVH * HD + g * HD: KVH * HD + (g + 1) * HD]
        wkv_s = np.concatenate([wk_s, wv_s], axis=1)
        wo_s = Wo[g * 256:(g + 1) * 256, :]
        im = {
            "xp": np.ascontiguousarray(
                x[b].T.reshape(8, 128, N_CHUNKS, N_CHUNK)
                .transpose(1, 2, 0, 3)),
            "wq": np.ascontiguousarray(
                wq_s.reshape(8, 128, 256).transpose(1, 0, 2)),
            "wkv": np.ascontiguousarray(
                wkv_s.reshape(8, 128, 128).transpose(1, 0, 2)),
            "wo": np.ascontiguousarray(
                wo_s.reshape(2, 128, DIM).transpose(1, 0, 2)
            ).astype(ml_dtypes.bfloat16),
            "tab": tab_b,
            "masks": masks_p,
            "blockind": _BLOCKIND,
        }
        in_maps.append(im)

    from concourse.bass_utils import run_bass_kernel_spmd
    res = run_bass_kernel_spmd(nc, in_maps, core_ids=list(range(8)), trace=False)

    out = np.zeros((B, S, DIM), dtype=np.float32)
    for c in range(8):
        out[c // 4] += res.results[c]["outT"].T
    return out
